# revision 42
# baseline (speedup 1.0000x reference)
"""Trainium2 Bass kernel for nn_EquivairantMultiheadAttention (sparse attention).

Contract: kernel(**inputs) takes the FULL unsharded numpy inputs (as produced by
setup_inputs()) and returns the FULL (B, N, COUT) float32 output.

Sharding: 8 cores = data-parallel over batch (2) x sequence-parallel over the
query dim n (4 slices of 512).

Architecture (v1, f16):
 - All PE inputs and gathered tables in float16; channel order permuted to
   d-major (ch' = d*8 + h) so per-head broadcasts have contiguous last dims
   (enables the DVE 2x 16-bit mode on every big elementwise op).
 - K and V rows stored interleaved in one DRAM table (1KB rows); one
   dma_gather per 1024 neighbors fetches both (half the descriptors).
 - pairwise_g neighbor rows are gathered on HOST (pure data selection) and
   streamed as a dense (512, 64, 6, 8) f16 tensor (h-expanded for 2x mode).
 - Scores: t1 = kg*Q elementwise + in-place halving tree over d (all f16 2x);
   t2 = pgx*G + tree; t3/b_k/b_l terms are constant over the softmax axis m
   and drop out. Softmax exp on the Act engine; normalization deferred.
 - Aggregation: per-m matmuls with identity lhsT accumulate E-scaled V rows
   in PSUM (replaces a DVE reduction tree); 1/Z applied on PSUM evac;
 - b_in folded into b_out on host: out = W_out@agg + (W_out@b_in + b_out).
"""

import math
import sys

import numpy as np

sys.path.insert(0, "/opt/trn_rl_repo")

B, N, M = 2, 2048, 64
C = 256  # CIN == COUT
H, D, POS = 8, 32, 6
NQ = 512  # queries per core
QB = 4  # query blocks of 128 per core
P = 128
NCORES = 8
INV_SQRT_D = 1.0 / math.sqrt(D)
NIC = 1024  # max idxs per dma_gather call (HW: larger crashes the exec unit)
CH2 = 2 * C  # combined K|V row width

_compiled = {}

# channel permutation: ch' = d*8 + h  <->  ch = h*32 + d
CHMAP = np.array([(cp % 8) * D + (cp // 8) for cp in range(C)], dtype=np.int64)


def build_bass():
    import concourse.bacc as bacc
    import concourse.mybir as mybir
    import concourse.tile as tile

    dt = mybir.dt
    nc = bacc.Bacc("TRN2", target_bir_lowering=False, debug=False,
                   enable_asserts=False, num_devices=NCORES)

    f16 = dt.float16
    f32 = dt.float32
    i16 = dt.int16

    # ---- DRAM inputs (per core) ----
    # cosT is rolled per-core so this core's NQ query columns are 0..NQ-1
    # (gather idx values are rotated to match on host).
    d_cosT = nc.dram_tensor("cosT", [2, P, N], f16, kind="ExternalInput")
    d_wkv = nc.dram_tensor("wkv", [2, P, CH2], f16, kind="ExternalInput")
    d_wq = nc.dram_tensor("wq", [2, P, C], f16, kind="ExternalInput")
    d_wl = nc.dram_tensor("wl", [2, P, H * POS], f16, kind="ExternalInput")
    d_wo = nc.dram_tensor("wo", [2, P, C], f16, kind="ExternalInput")
    d_bqv = nc.dram_tensor("bqv", [2, P, 1], f32, kind="ExternalInput")
    d_bqmat = nc.dram_tensor("bqmat", [P, C], f32, kind="ExternalInput")
    d_bomat = nc.dram_tensor("bomat", [P, C], f32, kind="ExternalInput")
    d_ident = nc.dram_tensor("ident", [P, P], f16, kind="ExternalInput")
    d_idxw = nc.dram_tensor("idxw", [P, QB, M * P // 16], i16,
                            kind="ExternalInput")
    d_pgx = nc.dram_tensor("pgx", [QB, P, M * POS * H], f16,
                           kind="ExternalInput")
    d_out = nc.dram_tensor("out", [QB, P, C], f32, kind="ExternalOutput")

    add = mybir.AluOpType.add
    mult = mybir.AluOpType.mult

    with tile.TileContext(nc) as tc:
        with (
            tc.tile_pool(name="const", bufs=1) as constp,
            tc.tile_pool(name="dram", bufs=1, space="DRAM") as dramp,
            tc.tile_pool(name="psum", bufs=2, space="PSUM") as psump,
            tc.tile_pool(name="evac", bufs=2) as evacp,
        ):
            # critical-path loads first: cosT gates the KV table -> gathers
            cosT = constp.tile([P, 2, N], f16)
            wkv = constp.tile([P, 2, CH2], f16)
            ident = constp.tile([P, P], f16)
            for cc in range(2):
                nc.sync.dma_start(cosT[:, cc, :], d_cosT[cc])
                nc.sync.dma_start(wkv[:, cc, :], d_wkv[cc])
            nc.sync.dma_start(ident[:], d_ident.ap())
            wq = constp.tile([P, 2, C], f16)
            wl = constp.tile([P, 2, H * POS], f16)
            wo = constp.tile([P, 2, C], f16)
            for cc in range(2):
                nc.sync.dma_start(wq[:, cc, :], d_wq[cc])
                nc.sync.dma_start(wl[:, cc, :], d_wl[cc])
                nc.sync.dma_start(wo[:, cc, :], d_wo[cc])
            bqv = constp.tile([P, 2, 1], f32)
            nc.sync.dma_start(bqv[:], d_bqv.ap().rearrange("c p one -> p c one"))
            bqmat = constp.tile([P, C], f32)
            bomat = constp.tile([P, C], f32)
            nc.sync.dma_start(bqmat[:], d_bqmat.ap())
            nc.sync.dma_start(bomat[:], d_bomat.ap())
            idxw = constp.tile([P, QB, M * P // 16], i16)
            nc.sync.dma_start(idxw[:], d_idxw.ap())
            nbias = constp.tile([P, 1], f32)
            nc.vector.memset(nbias[:], -4.0)

            # combined K|V rows (f16, permuted channels) in DRAM scratch
            kvdr = dramp.tile([N, CH2], f16)

            q_rows = constp.tile([P, QB, C], f16)
            g_rows = constp.tile([P, QB, H * POS], f16)

            with tc.tile_pool(name="prep", bufs=1) as prepp:
                # tiny matmuls advance the PE pstate ramp almost for free so
                # the KV-table matmuls run at speed
                psW = psump.tile([P, 2], f32, tag="pst", bufs=1)
                for i in range(12):
                    nc.tensor.matmul(psW[:], lhsT=ident[:], rhs=ident[:, 0:2],
                                     start=(i == 0), stop=(i == 11))
                JB = 4  # jt rows per batched kvdr write
                for jg in range(N // P // JB):
                    kv_sb = evacp.tile([P, JB, CH2], f16, tag="kvevac")
                    for j in range(JB):
                        jt = jg * JB + j
                        ps = psump.tile([P, CH2], f32, tag="ps", bufs=3)
                        for cc in range(2):
                            nc.tensor.matmul(
                                ps[:],
                                lhsT=cosT[:, cc, jt * P:(jt + 1) * P],
                                rhs=wkv[:, cc, :],
                                start=(cc == 0), stop=(cc == 1))
                        nc.vector.tensor_copy(kv_sb[:, j, :], ps[:])
                    nc.sync.dma_start(
                        kvdr[jg * JB * P:(jg + 1) * JB * P, :]
                        .rearrange("(j p) w -> p j w", j=JB),
                        kv_sb[:])

                for nt in range(QB):
                    ps = psump.tile([P, C], f32, tag="agg", bufs=2)
                    for cc in range(2):
                        nc.tensor.matmul(ps[:],
                                         lhsT=cosT[:, cc, nt * P:(nt + 1) * P],
                                         rhs=wq[:, cc, :],
                                         start=(cc == 0), stop=(cc == 1))
                    nc.vector.tensor_tensor(out=q_rows[:, nt, :], in0=ps[:],
                                            in1=bqmat[:], op=add)

                qvT = prepp.tile([P, 2, NQ], f16)
                for cc2 in range(2):
                    ps = psump.tile([P, NQ], f32, tag="agg", bufs=2)
                    for cc in range(2):
                        nc.tensor.matmul(ps[:],
                                         lhsT=wq[:, cc, cc2 * P:(cc2 + 1) * P],
                                         rhs=cosT[:, cc, 0:NQ],
                                         start=(cc == 0), stop=(cc == 1))
                    nc.vector.tensor_tensor(
                        out=qvT[:, cc2, :], in0=ps[:],
                        in1=bqv[:, cc2, :].broadcast_to([P, NQ]), op=add)
                for nt in range(QB):
                    ps = psump.tile([P, H * POS], f32, tag="agg", bufs=2)
                    for cc in range(2):
                        nc.tensor.matmul(ps[:],
                                         lhsT=qvT[:, cc, nt * P:(nt + 1) * P],
                                         rhs=wl[:, cc, :],
                                         start=(cc == 0), stop=(cc == 1))
                    nc.scalar.copy(g_rows[:, nt, :], ps[:])

            # ---- main loop: per query block ----
            with (
                tc.tile_pool(name="kvp", bufs=2) as kvp,
                tc.tile_pool(name="pgp", bufs=4) as pgp,
                tc.tile_pool(name="ap", bufs=4) as app,
                tc.tile_pool(name="small", bufs=2) as smallp,
            ):
                MH = M // 2

                def emit_half(qb, mh, kv, A, E, psA, psZ):
                    ms = slice(mh * MH, (mh + 1) * MH)
                    kg = (kv[:, ms, 0:C]
                          .rearrange("p m (d h) -> p m d h", h=H))
                    nc.vector.tensor_tensor(
                        out=kg, in0=kg,
                        in1=q_rows[:, qb, :][:, None, :]
                            .broadcast_to([P, MH, C])
                            .rearrange("p m (d h) -> p m d h", h=H),
                        op=mult)
                    dd = D // 2
                    while dd >= 1:
                        nc.vector.tensor_tensor(out=kg[:, :, 0:dd, :],
                                                in0=kg[:, :, 0:dd, :],
                                                in1=kg[:, :, dd:2 * dd, :],
                                                op=add)
                        dd //= 2
                    nc.vector.tensor_tensor(out=A[:, ms, :], in0=A[:, ms, :],
                                            in1=kg[:, :, 0, :], op=add)
                    nc.scalar.activation(
                        out=E[:, ms, :], in_=A[:, ms, :],
                        func=mybir.ActivationFunctionType.Exp,
                        scale=INV_SQRT_D, bias=nbias[:])
                    vg = (kv[:, ms, C:CH2]
                          .rearrange("p m (d h) -> p m d h", h=H))
                    nc.vector.tensor_tensor(
                        out=vg, in0=vg,
                        in1=E[:, ms, None, :].broadcast_to([P, MH, D, H]),
                        op=mult)
                    # Z via tiny identity matmuls: real work that also
                    # advances the PE pstate ramp before the wide aggregation
                    # burst (they only need E, not vg)
                    for mm in range(MH):
                        m0 = mh * MH + mm
                        nc.tensor.matmul(
                            psZ[:], lhsT=ident[:], rhs=E[:, m0, :],
                            start=(mh == 0 and mm == 0),
                            stop=(mh == 1 and mm == MH - 1),
                            skip_group_check=True)
                    for mm in range(MH // 2):
                        m0 = mh * MH + 2 * mm
                        nc.tensor.matmul(
                            psA[:], lhsT=ident[:],
                            rhs=kv[:, m0:m0 + 2, C:CH2],
                            start=(mh == 0 and mm == 0),
                            stop=(mh == 1 and mm == MH // 2 - 1),
                            skip_group_check=True)

                def emit_epilogue(qb, psA, psZ):
                    # fold psum halves, normalize (1/Z); transpose + out proj
                    rz = smallp.tile([P, H], f32, tag="rz", name=f"rz{qb}")
                    nc.vector.reciprocal(rz[:], psZ[:])
                    # normalize each PSUM half separately (TT may read at most
                    # one PSUM operand), then fold in SBUF (f16, 2x mode)
                    aggP = smallp.tile([P, 2, C], f16, tag="aggP",
                                       name=f"aggP{qb}")
                    for jj in range(2):
                        nc.vector.tensor_tensor(
                            out=aggP[:, jj, :].rearrange("p (d h) -> p d h",
                                                         h=H),
                            in0=psA[:, jj, :].rearrange("p (d h) -> p d h",
                                                        h=H),
                            in1=rz[:][:, None, :].broadcast_to([P, D, H]),
                            op=mult)
                    aggN = smallp.tile([P, C], f16, tag="aggN",
                                       name=f"aggN{qb}")
                    nc.vector.tensor_tensor(out=aggN[:], in0=aggP[:, 0, :],
                                            in1=aggP[:, 1, :], op=add)
                    aggT = smallp.tile([P, 2, P], f16, tag="aggT",
                                       name=f"aggT{qb}")
                    for cc in range(2):
                        pst = psump.tile([P, P], f16, tag="pst", bufs=1,
                                         name=f"pst{qb}_{cc}")
                        nc.tensor.transpose(pst[:],
                                            aggN[:, cc * P:(cc + 1) * P],
                                            ident[:])
                        nc.scalar.copy(aggT[:, cc, :], pst[:])
                    psO = psump.tile([P, C], f32, tag="pst", bufs=1,
                                     name=f"psO{qb}")
                    for cc in range(2):
                        nc.tensor.matmul(psO[:], lhsT=aggT[:, cc, :],
                                         rhs=wo[:, cc, :],
                                         start=(cc == 0), stop=(cc == 1))
                    out_sb = smallp.tile([P, C], f32, tag="outsb",
                                         name=f"outsb{qb}")
                    nc.vector.tensor_tensor(out=out_sb[:], in0=psO[:],
                                            in1=bomat[:], op=add)
                    nc.sync.dma_start(d_out[qb], out_sb[:])

                # stage first two pgx blocks before the gather stream starts
                pgx_tiles = {}
                pend = []  # deferred epilogue args (software pipelining)
                for qb in range(QB):
                    pgx_tiles[qb] = pgp.tile([P, M, POS, H], f16, tag="pgx",
                                             name=f"pgx{qb}")
                    nc.sync.dma_start(pgx_tiles[qb][:], d_pgx[qb])
                for qb in range(QB):
                    kv = kvp.tile([P, M, CH2], f16, tag="kv")
                    for k in range(M * P // NIC):
                        nc.gpsimd.dma_gather(
                            out_ap=kv[:, k * (NIC // P):(k + 1) * (NIC // P), :],
                            in_ap=kvdr[:],
                            idxs_ap=idxw[:, qb,
                                         k * (NIC // 16):(k + 1) * (NIC // 16)],
                            num_idxs=NIC, num_idxs_reg=NIC, elem_size=CH2)

                    # t2 -> A (rel-pos term); pgx layout [P, M, POS, H]
                    pgx = pgx_tiles[qb]
                    A = app.tile([P, M, H], f16, tag="A")
                    gb = (g_rows[:, qb, :]
                          .rearrange("p (pp h) -> p pp h", h=H)
                          [:, None, :, :].broadcast_to([P, M, POS, H]))
                    nc.vector.tensor_tensor(out=pgx[:], in0=pgx[:], in1=gb,
                                            op=mult)
                    nc.vector.tensor_tensor(out=pgx[:, :, 0:3, :],
                                            in0=pgx[:, :, 0:3, :],
                                            in1=pgx[:, :, 3:6, :], op=add)
                    nc.vector.tensor_tensor(out=pgx[:, :, 0:1, :],
                                            in0=pgx[:, :, 0:1, :],
                                            in1=pgx[:, :, 1:2, :], op=add)
                    nc.vector.tensor_tensor(out=A[:], in0=pgx[:, :, 0, :],
                                            in1=pgx[:, :, 2, :], op=add)

                    # per m-half: t1 scores, exp (fixed -4 bias keeps f16 E*V
                    # in range; softmax shift-invariant, 1/Z deferred), V
                    # scaling, and PE identity-accumulation — pipelined so PE
                    # aggregation of half 0 overlaps DVE scoring of half 1.
                    # The previous qblock's epilogue is emitted between the
                    # halves so its PSUM reads never stall this one's DVE.
                    E = smallp.tile([P, M, H], f16, tag="E")
                    psA = psump.tile([P, 2, C], f32, tag="agg")
                    psZ = psump.tile([P, H], f32, tag="aggz")
                    emit_half(qb, 0, kv, A, E, psA, psZ)
                    if pend:
                        emit_epilogue(*pend.pop())
                    emit_half(qb, 1, kv, A, E, psA, psZ)
                    pend.append((qb, psA, psZ))
                emit_epilogue(*pend.pop())

    nc.compile()
    return nc


def _wrap_idx(lst):
    """int16 list -> [128, len/16] wrapped (pos i -> [i%16, i//16]) and
    replicated across the 8 groups of 16 partitions."""
    n = lst.shape[0]
    w = np.empty((P, n // 16), np.int16)
    blk = lst.reshape(n // 16, 16).T  # [16, n/16]
    for g in range(8):
        w[g * 16:(g + 1) * 16, :] = blk
    return w


def make_core_inputs(pairwise_g, coset_functions, nbhd_idx,
                     W_q, b_q, W_k, W_l, v, W_in, b_in, W_out, b_out):
    pairwise_g = np.asarray(pairwise_g)
    coset_functions = np.asarray(coset_functions)
    nbhd_idx = np.asarray(nbhd_idx).astype(np.int64)
    W_q = np.asarray(W_q, np.float32)
    b_q = np.asarray(b_q, np.float32)
    W_k = np.asarray(W_k, np.float32)
    W_l = np.asarray(W_l, np.float32)
    v = np.asarray(v, np.float32)
    W_in = np.asarray(W_in, np.float32)
    b_in = np.asarray(b_in, np.float32)
    W_out = np.asarray(W_out, np.float32)
    b_out = np.asarray(b_out, np.float32)

    wq16 = np.ascontiguousarray(
        W_q[CHMAP].T.reshape(2, P, C).astype(np.float16))
    wkv16 = np.ascontiguousarray(
        np.concatenate([W_k[CHMAP].T, W_in[CHMAP].T], axis=1)
        .reshape(2, P, CH2).astype(np.float16))
    wo16 = np.ascontiguousarray(
        W_out.T[CHMAP].reshape(2, P, C).astype(np.float16))
    wl_full = np.zeros((C, H * POS), np.float32)
    for h in range(H):
        for p_ in range(POS):
            wl_full[h * D:(h + 1) * D, p_ * H + h] = W_l[h * D:(h + 1) * D, p_]
    wl16 = np.ascontiguousarray(
        wl_full[CHMAP].reshape(2, P, H * POS).astype(np.float16))
    bqv32 = np.ascontiguousarray(
        (b_q + v.reshape(C))[CHMAP].reshape(2, P, 1).astype(np.float32))
    bqmat = np.ascontiguousarray(
        np.broadcast_to(b_q[CHMAP], (P, C)).astype(np.float32))
    bomat = np.ascontiguousarray(
        np.broadcast_to(W_out @ b_in + b_out, (P, C)).astype(np.float32))
    ident16 = np.eye(P, dtype=np.float16)

    in_maps = []
    for core in range(NCORES):
        b = core // 4
        qs = (core % 4) * NQ
        # roll keys so this core's queries are columns 0..NQ-1 of cosT
        # (the compiled program slices cosT[:, :, 0:NQ] for Q projections);
        # gather idx values are rotated to match the rolled KV table.
        cosT = np.ascontiguousarray(
            np.roll(coset_functions[b], -qs, axis=0)
            .T.reshape(2, P, N).astype(np.float16))
        idx = nbhd_idx[b, qs:qs + NQ]  # [NQ, M]
        idx_rot = (idx - qs) % N

        idxw = np.empty((P, QB, M * P // 16), np.int16)
        for qb in range(QB):
            blk = idx_rot[qb * P:(qb + 1) * P]  # [P(n), M]
            lst = blk.T.reshape(M * P)  # m-major: pos i = m*128 + n
            idxw[:, qb, :] = _wrap_idx(lst.astype(np.int16))

        # host-gathered pairwise_g neighbor rows, h-expanded, f16
        pg_rows = pairwise_g[b, qs + np.arange(NQ)[:, None], idx]  # [NQ,M,POS]
        pgx = np.broadcast_to(pg_rows[:, :, :, None].astype(np.float16),
                              (NQ, M, POS, H))
        pgx = np.ascontiguousarray(pgx).reshape(QB, P, M * POS * H)

        in_maps.append({
            "cosT": cosT,
            "wkv": wkv16, "wq": wq16, "wl": wl16, "wo": wo16,
            "bqv": bqv32, "bqmat": bqmat, "bomat": bomat,
            "ident": ident16, "idxw": idxw, "pgx": pgx,
        })
    return in_maps


def assemble_output(results):
    out = np.empty((B, N, C), np.float32)
    for core in range(NCORES):
        b = core // 4
        qs = (core % 4) * NQ
        o = results[core]["out"]  # [QB, P, C]
        out[b, qs:qs + NQ] = o.reshape(NQ, C)
    return out


def kernel(pairwise_g, coset_functions, mask, nbhd_idx,
           W_q, b_q, W_k, b_k, W_l, b_l, u, v,
           W_in, b_in, W_out, b_out, **_unused):
    from concourse.bass_utils import run_bass_kernel_spmd

    if "nc" not in _compiled:
        _compiled["nc"] = build_bass()
    nc = _compiled["nc"]

    in_maps = make_core_inputs(pairwise_g, coset_functions, nbhd_idx,
                               W_q, b_q, W_k, W_l, v, W_in, b_in,
                               W_out, b_out)
    res = run_bass_kernel_spmd(nc, in_maps, core_ids=list(range(NCORES)))
    return assemble_output(res.results)


# revision 57
# speedup vs baseline: 1.0051x; 1.0051x over previous
"""Trainium2 Bass kernel for nn_EquivairantMultiheadAttention (sparse attention).

Contract: kernel(**inputs) takes the FULL unsharded numpy inputs (as produced by
setup_inputs()) and returns the FULL (B, N, COUT) float32 output.

Sharding: 8 cores = data-parallel over batch (2) x sequence-parallel over the
query dim n (4 slices of 512).

Architecture (v1, f16):
 - All PE inputs and gathered tables in float16; channel order permuted to
   d-major (ch' = d*8 + h) so per-head broadcasts have contiguous last dims
   (enables the DVE 2x 16-bit mode on every big elementwise op).
 - K and V rows stored interleaved in one DRAM table (1KB rows); one
   dma_gather per 1024 neighbors fetches both (half the descriptors).
 - pairwise_g neighbor rows are gathered on HOST (pure data selection) and
   streamed as a dense (512, 64, 6, 8) f16 tensor (h-expanded for 2x mode).
 - Scores: t1 = kg*Q elementwise + in-place halving tree over d (all f16 2x);
   t2 = pgx*G + tree; t3/b_k/b_l terms are constant over the softmax axis m
   and drop out. Softmax exp on the Act engine; normalization deferred.
 - Aggregation: per-m matmuls with identity lhsT accumulate E-scaled V rows
   in PSUM (replaces a DVE reduction tree); 1/Z applied on PSUM evac;
 - b_in folded into b_out on host: out = W_out@agg + (W_out@b_in + b_out).
"""

import math
import sys

import numpy as np

sys.path.insert(0, "/opt/trn_rl_repo")

B, N, M = 2, 2048, 64
C = 256  # CIN == COUT
H, D, POS = 8, 32, 6
NQ = 512  # queries per core
QB = 4  # query blocks of 128 per core
P = 128
NCORES = 8
INV_SQRT_D = 1.0 / math.sqrt(D)
NIC = 1024  # max idxs per dma_gather call (HW: larger crashes the exec unit)
CH2 = 2 * C  # combined K|V row width

_compiled = {}

# channel permutation: ch' = d*8 + h  <->  ch = h*32 + d
CHMAP = np.array([(cp % 8) * D + (cp // 8) for cp in range(C)], dtype=np.int64)


def build_bass():
    import concourse.bacc as bacc
    import concourse.mybir as mybir
    import concourse.tile as tile

    dt = mybir.dt
    nc = bacc.Bacc("TRN2", target_bir_lowering=False, debug=False,
                   enable_asserts=False, num_devices=NCORES)

    f16 = dt.float16
    f32 = dt.float32
    i16 = dt.int16

    # ---- DRAM inputs (per core) ----
    # cosT is rolled per-core so this core's NQ query columns are 0..NQ-1
    # (gather idx values are rotated to match on host).
    d_cosT = nc.dram_tensor("cosT", [2, P, N], f16, kind="ExternalInput")
    d_wkv = nc.dram_tensor("wkv", [2, P, CH2], f16, kind="ExternalInput")
    d_wq = nc.dram_tensor("wq", [2, P, C], f16, kind="ExternalInput")
    d_wl = nc.dram_tensor("wl", [2, P, H * POS], f16, kind="ExternalInput")
    d_wo = nc.dram_tensor("wo", [2, P, C], f16, kind="ExternalInput")
    d_bqv = nc.dram_tensor("bqv", [2, P, 1], f32, kind="ExternalInput")
    d_bqmat = nc.dram_tensor("bqmat", [P, C], f32, kind="ExternalInput")
    d_bomat = nc.dram_tensor("bomat", [P, C], f32, kind="ExternalInput")
    d_ident = nc.dram_tensor("ident", [P, P], f16, kind="ExternalInput")
    d_idxw = nc.dram_tensor("idxw", [P, QB, M * P // 16], i16,
                            kind="ExternalInput")
    d_pgx = nc.dram_tensor("pgx", [QB, P, M * POS * H], f16,
                           kind="ExternalInput")
    d_out = nc.dram_tensor("out", [QB, P, C], f32, kind="ExternalOutput")

    add = mybir.AluOpType.add
    mult = mybir.AluOpType.mult

    with tile.TileContext(nc) as tc:
        with (
            tc.tile_pool(name="const", bufs=1) as constp,
            tc.tile_pool(name="dram", bufs=1, space="DRAM") as dramp,
            tc.tile_pool(name="psum", bufs=2, space="PSUM") as psump,
            tc.tile_pool(name="evac", bufs=2) as evacp,
        ):
            # critical-path loads first: cosT gates the KV table -> gathers
            cosT = constp.tile([P, 2, N], f16)
            wkv = constp.tile([P, 2, CH2], f16)
            ident = constp.tile([P, P], f16)
            for cc in range(2):
                nc.sync.dma_start(cosT[:, cc, :], d_cosT[cc])
                nc.sync.dma_start(wkv[:, cc, :], d_wkv[cc])
            nc.sync.dma_start(ident[:], d_ident.ap())
            wq = constp.tile([P, 2, C], f16)
            wl = constp.tile([P, 2, H * POS], f16)
            wo = constp.tile([P, 2, C], f16)
            for cc in range(2):
                nc.sync.dma_start(wq[:, cc, :], d_wq[cc])
                nc.sync.dma_start(wl[:, cc, :], d_wl[cc])
                nc.sync.dma_start(wo[:, cc, :], d_wo[cc])
            bqv = constp.tile([P, 2, 1], f32)
            nc.sync.dma_start(bqv[:], d_bqv.ap().rearrange("c p one -> p c one"))
            bqmat = constp.tile([P, C], f32)
            bomat = constp.tile([P, C], f32)
            nc.sync.dma_start(bqmat[:], d_bqmat.ap())
            nc.sync.dma_start(bomat[:], d_bomat.ap())
            idxw = constp.tile([P, QB, M * P // 16], i16)
            nc.sync.dma_start(idxw[:], d_idxw.ap())
            nbias = constp.tile([P, 1], f32)
            nc.vector.memset(nbias[:], -4.0)

            # combined K|V rows (f16, permuted channels) in DRAM scratch
            kvdr = dramp.tile([N, CH2], f16)

            q_rows = constp.tile([P, QB, C], f16)
            g_rows = constp.tile([P, QB, H * POS], f16)

            with tc.tile_pool(name="prep", bufs=1) as prepp:
                # tiny matmuls advance the PE pstate ramp almost for free so
                # the KV-table matmuls run at speed
                psW = psump.tile([P, 2], f32, tag="pst", bufs=1)
                for i in range(12):
                    nc.tensor.matmul(psW[:], lhsT=ident[:], rhs=ident[:, 0:2],
                                     start=(i == 0), stop=(i == 11))
                JB = 4  # jt rows per batched kvdr write
                for jg in range(N // P // JB):
                    kv_sb = evacp.tile([P, JB, CH2], f16, tag="kvevac")
                    for j in range(JB):
                        jt = jg * JB + j
                        ps = psump.tile([P, CH2], f32, tag="ps", bufs=3)
                        for cc in range(2):
                            nc.tensor.matmul(
                                ps[:],
                                lhsT=cosT[:, cc, jt * P:(jt + 1) * P],
                                rhs=wkv[:, cc, :],
                                start=(cc == 0), stop=(cc == 1))
                        nc.vector.tensor_copy(kv_sb[:, j, :], ps[:])
                    nc.sync.dma_start(
                        kvdr[jg * JB * P:(jg + 1) * JB * P, :]
                        .rearrange("(j p) w -> p j w", j=JB),
                        kv_sb[:])

                for nt in range(QB):
                    ps = psump.tile([P, C], f32, tag="agg", bufs=2)
                    for cc in range(2):
                        nc.tensor.matmul(ps[:],
                                         lhsT=cosT[:, cc, nt * P:(nt + 1) * P],
                                         rhs=wq[:, cc, :],
                                         start=(cc == 0), stop=(cc == 1))
                    nc.vector.tensor_tensor(out=q_rows[:, nt, :], in0=ps[:],
                                            in1=bqmat[:], op=add)

                qvT = prepp.tile([P, 2, NQ], f16)
                for cc2 in range(2):
                    ps = psump.tile([P, NQ], f32, tag="agg", bufs=2)
                    for cc in range(2):
                        nc.tensor.matmul(ps[:],
                                         lhsT=wq[:, cc, cc2 * P:(cc2 + 1) * P],
                                         rhs=cosT[:, cc, 0:NQ],
                                         start=(cc == 0), stop=(cc == 1))
                    nc.vector.tensor_tensor(
                        out=qvT[:, cc2, :], in0=ps[:],
                        in1=bqv[:, cc2, :].broadcast_to([P, NQ]), op=add)
                for nt in range(QB):
                    ps = psump.tile([P, H * POS], f32, tag="agg", bufs=2)
                    for cc in range(2):
                        nc.tensor.matmul(ps[:],
                                         lhsT=qvT[:, cc, nt * P:(nt + 1) * P],
                                         rhs=wl[:, cc, :],
                                         start=(cc == 0), stop=(cc == 1))
                    nc.scalar.copy(g_rows[:, nt, :], ps[:])

            # ---- main loop: per query block ----
            with (
                tc.tile_pool(name="kvp", bufs=4) as kvp,
                tc.tile_pool(name="pgp", bufs=4) as pgp,
                tc.tile_pool(name="ap", bufs=4) as app,
                tc.tile_pool(name="small", bufs=2) as smallp,
            ):
                MH = M // 2

                def emit_score(qb, mh, kvh, A, E):
                    ms = slice(mh * MH, (mh + 1) * MH)
                    kg = (kvh[:, :, 0:C]
                          .rearrange("p m (d h) -> p m d h", h=H))
                    nc.vector.tensor_tensor(
                        out=kg, in0=kg,
                        in1=q_rows[:, qb, :][:, None, :]
                            .broadcast_to([P, MH, C])
                            .rearrange("p m (d h) -> p m d h", h=H),
                        op=mult)
                    dd = D // 2
                    while dd >= 1:
                        nc.vector.tensor_tensor(out=kg[:, :, 0:dd, :],
                                                in0=kg[:, :, 0:dd, :],
                                                in1=kg[:, :, dd:2 * dd, :],
                                                op=add)
                        dd //= 2
                    nc.vector.tensor_tensor(out=A[:, ms, :], in0=A[:, ms, :],
                                            in1=kg[:, :, 0, :], op=add)
                    nc.scalar.activation(
                        out=E[:, ms, :], in_=A[:, ms, :],
                        func=mybir.ActivationFunctionType.Exp,
                        scale=INV_SQRT_D, bias=nbias[:])

                def emit_value(qb, mh, kvh, E, psA, psZ):
                    ms = slice(mh * MH, (mh + 1) * MH)
                    vg = (kvh[:, :, C:CH2]
                          .rearrange("p m (d h) -> p m d h", h=H))
                    nc.vector.tensor_tensor(
                        out=vg, in0=vg,
                        in1=E[:, ms, None, :].broadcast_to([P, MH, D, H]),
                        op=mult)
                    # Z via tiny identity matmuls: real work that also
                    # advances the PE pstate ramp before the wide aggregation
                    # burst (they only need E, not vg)
                    for mm in range(MH):
                        m0 = mh * MH + mm
                        nc.tensor.matmul(
                            psZ[:], lhsT=ident[:], rhs=E[:, m0, :],
                            start=(mh == 0 and mm == 0),
                            stop=(mh == 1 and mm == MH - 1),
                            skip_group_check=True)
                    for mm in range(MH):
                        nc.tensor.matmul(
                            psA[:], lhsT=ident[:],
                            rhs=kvh[:, mm, C:CH2],
                            start=(mh == 0 and mm == 0),
                            stop=(mh == 1 and mm == MH - 1),
                            skip_group_check=True)

                def emit_epilogue(qb, psA, psZ):
                    # fold psum halves, normalize (1/Z); transpose + out proj
                    rz = smallp.tile([P, H], f32, tag="rz", name=f"rz{qb}")
                    nc.vector.reciprocal(rz[:], psZ[:])
                    aggN = smallp.tile([P, C], f16, tag="aggN",
                                       name=f"aggN{qb}")
                    nc.vector.tensor_tensor(
                        out=aggN[:].rearrange("p (d h) -> p d h", h=H),
                        in0=psA[:].rearrange("p (d h) -> p d h", h=H),
                        in1=rz[:][:, None, :].broadcast_to([P, D, H]),
                        op=mult)
                    aggT = smallp.tile([P, 2, P], f16, tag="aggT",
                                       name=f"aggT{qb}")
                    for cc in range(2):
                        pst = psump.tile([P, P], f16, tag="pst", bufs=1,
                                         name=f"pst{qb}_{cc}")
                        nc.tensor.transpose(pst[:],
                                            aggN[:, cc * P:(cc + 1) * P],
                                            ident[:])
                        nc.scalar.copy(aggT[:, cc, :], pst[:])
                    psO = psump.tile([P, C], f32, tag="pst", bufs=1,
                                     name=f"psO{qb}")
                    for cc in range(2):
                        nc.tensor.matmul(psO[:], lhsT=aggT[:, cc, :],
                                         rhs=wo[:, cc, :],
                                         start=(cc == 0), stop=(cc == 1))
                    out_sb = smallp.tile([P, C], f32, tag="outsb",
                                         name=f"outsb{qb}")
                    nc.vector.tensor_tensor(out=out_sb[:], in0=psO[:],
                                            in1=bomat[:], op=add)
                    nc.sync.dma_start(d_out[qb], out_sb[:])

                pgx_tiles = {}
                pend = []  # deferred epilogue args (software pipelining)
                for qb in range(QB):
                    pgx_tiles[qb] = pgp.tile([P, M, POS, H], f16, tag="pgx",
                                             name=f"pgx{qb}")
                    nc.sync.dma_start(pgx_tiles[qb][:], d_pgx[qb])
                for qb in range(QB):
                    kvh2 = []
                    for mh in range(2):
                        kvh = kvp.tile([P, MH, CH2], f16, tag="kv",
                                       name=f"kv{qb}_{mh}")
                        kvh2.append(kvh)
                        for kk in range(4):
                            k = mh * 4 + kk
                            nc.gpsimd.dma_gather(
                                out_ap=kvh[:, kk * (NIC // P):
                                           (kk + 1) * (NIC // P), :],
                                in_ap=kvdr[:],
                                idxs_ap=idxw[:, qb, k * (NIC // 16):
                                             (k + 1) * (NIC // 16)],
                                num_idxs=NIC, num_idxs_reg=NIC,
                                elem_size=CH2)

                    # t2 -> A (rel-pos term); pgx layout [P, M, POS, H]
                    pgx = pgx_tiles[qb]
                    A = app.tile([P, M, H], f16, tag="A")
                    gb = (g_rows[:, qb, :]
                          .rearrange("p (pp h) -> p pp h", h=H)
                          [:, None, :, :].broadcast_to([P, M, POS, H]))
                    nc.vector.tensor_tensor(out=pgx[:], in0=pgx[:], in1=gb,
                                            op=mult)
                    nc.vector.tensor_tensor(out=pgx[:, :, 0:3, :],
                                            in0=pgx[:, :, 0:3, :],
                                            in1=pgx[:, :, 3:6, :], op=add)
                    nc.vector.tensor_tensor(out=pgx[:, :, 0:1, :],
                                            in0=pgx[:, :, 0:1, :],
                                            in1=pgx[:, :, 1:2, :], op=add)
                    nc.vector.tensor_tensor(out=A[:], in0=pgx[:, :, 0, :],
                                            in1=pgx[:, :, 2, :], op=add)

                    # per m-half: t1 scores, exp (fixed -4 bias keeps f16 E*V
                    # in range; softmax shift-invariant, 1/Z deferred), V
                    # scaling, and PE identity-accumulation — pipelined so PE
                    # aggregation of half 0 overlaps DVE scoring of half 1.
                    # The previous qblock's epilogue is emitted between the
                    # halves so its PSUM reads never stall this one's DVE.
                    E = smallp.tile([P, M, H], f16, tag="E")
                    psA = psump.tile([P, C], f32, tag="agg")
                    psZ = psump.tile([P, H], f32, tag="aggz")
                    emit_score(qb, 0, kvh2[0], A, E)
                    emit_score(qb, 1, kvh2[1], A, E)
                    emit_value(qb, 0, kvh2[0], E, psA, psZ)
                    if pend:
                        emit_epilogue(*pend.pop())
                    emit_value(qb, 1, kvh2[1], E, psA, psZ)
                    pend.append((qb, psA, psZ))
                emit_epilogue(*pend.pop())

    nc.compile()
    return nc


def _wrap_idx(lst):
    """int16 list -> [128, len/16] wrapped (pos i -> [i%16, i//16]) and
    replicated across the 8 groups of 16 partitions."""
    n = lst.shape[0]
    w = np.empty((P, n // 16), np.int16)
    blk = lst.reshape(n // 16, 16).T  # [16, n/16]
    for g in range(8):
        w[g * 16:(g + 1) * 16, :] = blk
    return w


def make_core_inputs(pairwise_g, coset_functions, nbhd_idx,
                     W_q, b_q, W_k, W_l, v, W_in, b_in, W_out, b_out):
    pairwise_g = np.asarray(pairwise_g)
    coset_functions = np.asarray(coset_functions)
    nbhd_idx = np.asarray(nbhd_idx).astype(np.int64)
    W_q = np.asarray(W_q, np.float32)
    b_q = np.asarray(b_q, np.float32)
    W_k = np.asarray(W_k, np.float32)
    W_l = np.asarray(W_l, np.float32)
    v = np.asarray(v, np.float32)
    W_in = np.asarray(W_in, np.float32)
    b_in = np.asarray(b_in, np.float32)
    W_out = np.asarray(W_out, np.float32)
    b_out = np.asarray(b_out, np.float32)

    wq16 = np.ascontiguousarray(
        W_q[CHMAP].T.reshape(2, P, C).astype(np.float16))
    wkv16 = np.ascontiguousarray(
        np.concatenate([W_k[CHMAP].T, W_in[CHMAP].T], axis=1)
        .reshape(2, P, CH2).astype(np.float16))
    wo16 = np.ascontiguousarray(
        W_out.T[CHMAP].reshape(2, P, C).astype(np.float16))
    wl_full = np.zeros((C, H * POS), np.float32)
    for h in range(H):
        for p_ in range(POS):
            wl_full[h * D:(h + 1) * D, p_ * H + h] = W_l[h * D:(h + 1) * D, p_]
    wl16 = np.ascontiguousarray(
        wl_full[CHMAP].reshape(2, P, H * POS).astype(np.float16))
    bqv32 = np.ascontiguousarray(
        (b_q + v.reshape(C))[CHMAP].reshape(2, P, 1).astype(np.float32))
    bqmat = np.ascontiguousarray(
        np.broadcast_to(b_q[CHMAP], (P, C)).astype(np.float32))
    bomat = np.ascontiguousarray(
        np.broadcast_to(W_out @ b_in + b_out, (P, C)).astype(np.float32))
    ident16 = np.eye(P, dtype=np.float16)

    in_maps = []
    for core in range(NCORES):
        b = core // 4
        qs = (core % 4) * NQ
        # roll keys so this core's queries are columns 0..NQ-1 of cosT
        # (the compiled program slices cosT[:, :, 0:NQ] for Q projections);
        # gather idx values are rotated to match the rolled KV table.
        cosT = np.ascontiguousarray(
            np.roll(coset_functions[b], -qs, axis=0)
            .T.reshape(2, P, N).astype(np.float16))
        idx = nbhd_idx[b, qs:qs + NQ]  # [NQ, M]
        idx_rot = (idx - qs) % N

        idxw = np.empty((P, QB, M * P // 16), np.int16)
        for qb in range(QB):
            blk = idx_rot[qb * P:(qb + 1) * P]  # [P(n), M]
            lst = blk.T.reshape(M * P)  # m-major: pos i = m*128 + n
            idxw[:, qb, :] = _wrap_idx(lst.astype(np.int16))

        # host-gathered pairwise_g neighbor rows, h-expanded, f16
        pg_rows = pairwise_g[b, qs + np.arange(NQ)[:, None], idx]  # [NQ,M,POS]
        pgx = np.broadcast_to(pg_rows[:, :, :, None].astype(np.float16),
                              (NQ, M, POS, H))
        pgx = np.ascontiguousarray(pgx).reshape(QB, P, M * POS * H)

        in_maps.append({
            "cosT": cosT,
            "wkv": wkv16, "wq": wq16, "wl": wl16, "wo": wo16,
            "bqv": bqv32, "bqmat": bqmat, "bomat": bomat,
            "ident": ident16, "idxw": idxw, "pgx": pgx,
        })
    return in_maps


def assemble_output(results):
    out = np.empty((B, N, C), np.float32)
    for core in range(NCORES):
        b = core // 4
        qs = (core % 4) * NQ
        o = results[core]["out"]  # [QB, P, C]
        out[b, qs:qs + NQ] = o.reshape(NQ, C)
    return out


def kernel(pairwise_g, coset_functions, mask, nbhd_idx,
           W_q, b_q, W_k, b_k, W_l, b_l, u, v,
           W_in, b_in, W_out, b_out, **_unused):
    from concourse.bass_utils import run_bass_kernel_spmd

    if "nc" not in _compiled:
        _compiled["nc"] = build_bass()
    nc = _compiled["nc"]

    in_maps = make_core_inputs(pairwise_g, coset_functions, nbhd_idx,
                               W_q, b_q, W_k, W_l, v, W_in, b_in,
                               W_out, b_out)
    res = run_bass_kernel_spmd(nc, in_maps, core_ids=list(range(NCORES)))
    return assemble_output(res.results)


# revision 66
# speedup vs baseline: 1.0790x; 1.0735x over previous
"""Trainium2 Bass kernel for nn_EquivairantMultiheadAttention (sparse attention).

Contract: kernel(**inputs) takes the FULL unsharded numpy inputs (as produced by
setup_inputs()) and returns the FULL (B, N, COUT) float32 output.

Sharding: 8 cores = data-parallel over batch (2) x sequence-parallel over the
query dim n (4 slices of 512).

Architecture (v1, f16):
 - All PE inputs and gathered tables in float16; channel order permuted to
   d-major (ch' = d*8 + h) so per-head broadcasts have contiguous last dims
   (enables the DVE 2x 16-bit mode on every big elementwise op).
 - K and V rows stored interleaved in one DRAM table (1KB rows); one
   dma_gather per 1024 neighbors fetches both (half the descriptors).
 - pairwise_g neighbor rows are gathered on HOST (pure data selection) and
   streamed as a dense (512, 64, 6, 8) f16 tensor (h-expanded for 2x mode).
 - Scores: t1 = kg*Q elementwise + in-place halving tree over d (all f16 2x);
   t2 = pgx*G + tree; t3/b_k/b_l terms are constant over the softmax axis m
   and drop out. Softmax exp on the Act engine; normalization deferred.
 - Aggregation: per-m matmuls with identity lhsT accumulate E-scaled V rows
   in PSUM (replaces a DVE reduction tree); 1/Z applied on PSUM evac;
 - b_in folded into b_out on host: out = W_out@agg + (W_out@b_in + b_out).
"""

import math
import sys

import numpy as np

sys.path.insert(0, "/opt/trn_rl_repo")

B, N, M = 2, 2048, 64
C = 256  # CIN == COUT
H, D, POS = 8, 32, 6
NQ = 512  # queries per core
QB = 4  # query blocks of 128 per core
P = 128
NCORES = 8
INV_SQRT_D = 1.0 / math.sqrt(D)
NIC = 1024  # max idxs per dma_gather call (HW: larger crashes the exec unit)
CH2 = 2 * C  # combined K|V row width

_compiled = {}

# channel permutation: ch' = d*8 + h  <->  ch = h*32 + d
CHMAP = np.array([(cp % 8) * D + (cp // 8) for cp in range(C)], dtype=np.int64)


def build_bass():
    import concourse.bacc as bacc
    import concourse.mybir as mybir
    import concourse.tile as tile

    dt = mybir.dt
    nc = bacc.Bacc("TRN2", target_bir_lowering=False, debug=False,
                   enable_asserts=False, num_devices=NCORES)

    f16 = dt.float16
    f32 = dt.float32
    i16 = dt.int16

    # ---- DRAM inputs (per core) ----
    # cosT is rolled per-core so this core's NQ query columns are 0..NQ-1
    # (gather idx values are rotated to match on host).
    d_cosT = nc.dram_tensor("cosT", [2, P, N], f16, kind="ExternalInput")
    d_wkv = nc.dram_tensor("wkv", [2, P, CH2], f16, kind="ExternalInput")
    d_wq = nc.dram_tensor("wq", [2, P, C], f16, kind="ExternalInput")
    d_wl = nc.dram_tensor("wl", [2, P, H * POS], f16, kind="ExternalInput")
    d_wo = nc.dram_tensor("wo", [2, P, C], f16, kind="ExternalInput")
    d_bqv = nc.dram_tensor("bqv", [2, P, 1], f32, kind="ExternalInput")
    d_bqmat = nc.dram_tensor("bqmat", [P, C], f32, kind="ExternalInput")
    d_bomat = nc.dram_tensor("bomat", [P, C], f32, kind="ExternalInput")
    d_ident = nc.dram_tensor("ident", [P, P], f16, kind="ExternalInput")
    d_idxw = nc.dram_tensor("idxw", [P, QB, M * P // 16], i16,
                            kind="ExternalInput")
    d_pgx = nc.dram_tensor("pgx", [QB, P, M * POS * H], f16,
                           kind="ExternalInput")
    d_out = nc.dram_tensor("out", [QB, P, C], f32, kind="ExternalOutput")

    add = mybir.AluOpType.add
    mult = mybir.AluOpType.mult

    with tile.TileContext(nc) as tc:
        with (
            tc.tile_pool(name="const", bufs=1) as constp,
            tc.tile_pool(name="dram", bufs=1, space="DRAM") as dramp,
            tc.tile_pool(name="psum", bufs=2, space="PSUM") as psump,
            tc.tile_pool(name="evac", bufs=2) as evacp,
        ):
            # critical-path loads first: cosT gates the KV table -> gathers
            cosT = constp.tile([P, 2, N], f16)
            wkv = constp.tile([P, 2, CH2], f16)
            ident = constp.tile([P, P], f16)
            for cc in range(2):
                nc.sync.dma_start(cosT[:, cc, :], d_cosT[cc])
                nc.sync.dma_start(wkv[:, cc, :], d_wkv[cc])
            nc.sync.dma_start(ident[:], d_ident.ap())
            wq = constp.tile([P, 2, C], f16)
            wl = constp.tile([P, 2, H * POS], f16)
            wo = constp.tile([P, 2, C], f16)
            for cc in range(2):
                nc.sync.dma_start(wq[:, cc, :], d_wq[cc])
                nc.sync.dma_start(wl[:, cc, :], d_wl[cc])
                nc.sync.dma_start(wo[:, cc, :], d_wo[cc])
            bqv = constp.tile([P, 2, 1], f32)
            nc.sync.dma_start(bqv[:], d_bqv.ap().rearrange("c p one -> p c one"))
            bqmat = constp.tile([P, C], f32)
            bomat = constp.tile([P, C], f32)
            nc.sync.dma_start(bqmat[:], d_bqmat.ap())
            nc.sync.dma_start(bomat[:], d_bomat.ap())
            idxw = constp.tile([P, QB, M * P // 16], i16)
            nc.sync.dma_start(idxw[:], d_idxw.ap())
            nbias = constp.tile([P, 1], f32)
            nc.vector.memset(nbias[:], -4.0)

            # combined K|V rows (f16, permuted channels) in DRAM scratch
            kvdr = dramp.tile([N, CH2], f16)

            q_rows = constp.tile([P, QB, C], f16)
            g_rows = constp.tile([P, QB, H * POS], f16)

            with tc.tile_pool(name="prep", bufs=1) as prepp:
                # tiny matmuls advance the PE pstate ramp almost for free so
                # the KV-table matmuls run at speed
                psW = psump.tile([P, 2], f32, tag="pst", bufs=1)
                for i in range(12):
                    nc.tensor.matmul(psW[:], lhsT=ident[:], rhs=ident[:, 0:2],
                                     start=(i == 0), stop=(i == 11))
                JB = 4  # jt rows per batched kvdr write
                for jg in range(N // P // JB):
                    kv_sb = evacp.tile([P, JB, CH2], f16, tag="kvevac")
                    for j in range(JB):
                        jt = jg * JB + j
                        ps = psump.tile([P, CH2], f32, tag="ps", bufs=3)
                        for cc in range(2):
                            nc.tensor.matmul(
                                ps[:],
                                lhsT=cosT[:, cc, jt * P:(jt + 1) * P],
                                rhs=wkv[:, cc, :],
                                start=(cc == 0), stop=(cc == 1))
                        nc.vector.tensor_copy(kv_sb[:, j, :], ps[:])
                    nc.sync.dma_start(
                        kvdr[jg * JB * P:(jg + 1) * JB * P, :]
                        .rearrange("(j p) w -> p j w", j=JB),
                        kv_sb[:])

                for nt in range(QB):
                    ps = psump.tile([P, C], f32, tag="agg", bufs=2)
                    for cc in range(2):
                        nc.tensor.matmul(ps[:],
                                         lhsT=cosT[:, cc, nt * P:(nt + 1) * P],
                                         rhs=wq[:, cc, :],
                                         start=(cc == 0), stop=(cc == 1))
                    nc.vector.tensor_tensor(out=q_rows[:, nt, :], in0=ps[:],
                                            in1=bqmat[:], op=add)

                qvT = prepp.tile([P, 2, NQ], f16)
                for cc2 in range(2):
                    ps = psump.tile([P, NQ], f32, tag="agg", bufs=2)
                    for cc in range(2):
                        nc.tensor.matmul(ps[:],
                                         lhsT=wq[:, cc, cc2 * P:(cc2 + 1) * P],
                                         rhs=cosT[:, cc, 0:NQ],
                                         start=(cc == 0), stop=(cc == 1))
                    nc.vector.tensor_tensor(
                        out=qvT[:, cc2, :], in0=ps[:],
                        in1=bqv[:, cc2, :].broadcast_to([P, NQ]), op=add)
                for nt in range(QB):
                    ps = psump.tile([P, H * POS], f32, tag="agg", bufs=2)
                    for cc in range(2):
                        nc.tensor.matmul(ps[:],
                                         lhsT=qvT[:, cc, nt * P:(nt + 1) * P],
                                         rhs=wl[:, cc, :],
                                         start=(cc == 0), stop=(cc == 1))
                    nc.scalar.copy(g_rows[:, nt, :], ps[:])

            # ---- main loop: per query block ----
            with (
                tc.tile_pool(name="kvp", bufs=4) as kvp,
                tc.tile_pool(name="pgp", bufs=4) as pgp,
                tc.tile_pool(name="ap", bufs=4) as app,
                tc.tile_pool(name="small", bufs=2) as smallp,
            ):
                MH = M // 2

                def emit_score(qb, mh, kvh, A, E):
                    # chunked at gather-call granularity (8 m per call) so
                    # scoring starts as soon as the first call lands
                    ms = slice(mh * MH, (mh + 1) * MH)
                    CHK = 16
                    for ck in range(MH // CHK):
                        cs0 = mh * MH + ck * CHK
                        kg = (kvh[:, ck * CHK:(ck + 1) * CHK, 0:C]
                              .rearrange("p m (d h) -> p m d h", h=H))
                        nc.vector.tensor_tensor(
                            out=kg, in0=kg,
                            in1=q_rows[:, qb, :][:, None, :]
                                .broadcast_to([P, CHK, C])
                                .rearrange("p m (d h) -> p m d h", h=H),
                            op=mult)
                        dd = D // 2
                        while dd >= 1:
                            nc.vector.tensor_tensor(out=kg[:, :, 0:dd, :],
                                                    in0=kg[:, :, 0:dd, :],
                                                    in1=kg[:, :, dd:2 * dd, :],
                                                    op=add)
                            dd //= 2
                        nc.vector.tensor_tensor(
                            out=A[:, cs0:cs0 + CHK, :],
                            in0=A[:, cs0:cs0 + CHK, :],
                            in1=kg[:, :, 0, :], op=add)
                    nc.scalar.activation(
                        out=E[:, ms, :], in_=A[:, ms, :],
                        func=mybir.ActivationFunctionType.Exp,
                        scale=INV_SQRT_D, bias=nbias[:])

                def emit_value(qb, mh, kvh, E, psA, psZ):
                    # Z via tiny identity matmuls: real work that also
                    # advances the PE pstate ramp (they only need E, not vg)
                    for mm in range(MH):
                        m0 = mh * MH + mm
                        nc.tensor.matmul(
                            psZ[:], lhsT=ident[:], rhs=E[:, m0, :],
                            start=(mh == 0 and mm == 0),
                            stop=(mh == 1 and mm == MH - 1),
                            skip_group_check=True)
                    # chunk the E-scaling so the first aggregation matmuls
                    # start ~3us earlier and PE stays continuously busy
                    CHK = 8
                    for ck in range(MH // CHK):
                        cs0 = mh * MH + ck * CHK
                        vgc = (kvh[:, ck * CHK:(ck + 1) * CHK, C:CH2]
                               .rearrange("p m (d h) -> p m d h", h=H))
                        nc.vector.tensor_tensor(
                            out=vgc, in0=vgc,
                            in1=E[:, cs0:cs0 + CHK, None, :]
                                .broadcast_to([P, CHK, D, H]),
                            op=mult)
                        for mm in range(ck * CHK, (ck + 1) * CHK):
                            nc.tensor.matmul(
                                psA[:], lhsT=ident[:],
                                rhs=kvh[:, mm, C:CH2],
                                start=(mh == 0 and mm == 0),
                                stop=(mh == 1 and mm == MH - 1),
                                skip_group_check=True)

                def emit_epilogue(qb, psA, psZ):
                    # fold psum halves, normalize (1/Z); transpose + out proj
                    rz = smallp.tile([P, H], f32, tag="rz", name=f"rz{qb}")
                    nc.vector.reciprocal(rz[:], psZ[:])
                    aggN = smallp.tile([P, C], f16, tag="aggN",
                                       name=f"aggN{qb}")
                    nc.vector.tensor_tensor(
                        out=aggN[:].rearrange("p (d h) -> p d h", h=H),
                        in0=psA[:].rearrange("p (d h) -> p d h", h=H),
                        in1=rz[:][:, None, :].broadcast_to([P, D, H]),
                        op=mult)
                    aggT = smallp.tile([P, 2, P], f16, tag="aggT",
                                       name=f"aggT{qb}")
                    for cc in range(2):
                        pst = psump.tile([P, P], f16, tag="pst", bufs=1,
                                         name=f"pst{qb}_{cc}")
                        nc.tensor.transpose(pst[:],
                                            aggN[:, cc * P:(cc + 1) * P],
                                            ident[:])
                        nc.scalar.copy(aggT[:, cc, :], pst[:])
                    psO = psump.tile([P, C], f32, tag="pst", bufs=1,
                                     name=f"psO{qb}")
                    for cc in range(2):
                        nc.tensor.matmul(psO[:], lhsT=aggT[:, cc, :],
                                         rhs=wo[:, cc, :],
                                         start=(cc == 0), stop=(cc == 1))
                    out_sb = smallp.tile([P, C], f32, tag="outsb",
                                         name=f"outsb{qb}")
                    nc.vector.tensor_tensor(out=out_sb[:], in0=psO[:],
                                            in1=bomat[:], op=add)
                    nc.sync.dma_start(d_out[qb], out_sb[:])

                pgx_tiles = {}
                pend = []  # deferred epilogue args (software pipelining)
                for qb in range(QB):
                    pgx_tiles[qb] = pgp.tile([P, M, POS, H], f16, tag="pgx",
                                             name=f"pgx{qb}")
                    nc.sync.dma_start(pgx_tiles[qb][:], d_pgx[qb])
                for qb in range(QB):
                    kvh2 = []
                    for mh in range(2):
                        kvh = kvp.tile([P, MH, CH2], f16, tag="kv",
                                       name=f"kv{qb}_{mh}")
                        kvh2.append(kvh)
                        for kk in range(4):
                            k = mh * 4 + kk
                            nc.gpsimd.dma_gather(
                                out_ap=kvh[:, kk * (NIC // P):
                                           (kk + 1) * (NIC // P), :],
                                in_ap=kvdr[:],
                                idxs_ap=idxw[:, qb, k * (NIC // 16):
                                             (k + 1) * (NIC // 16)],
                                num_idxs=NIC, num_idxs_reg=NIC,
                                elem_size=CH2)

                    # t2 -> A (rel-pos term); pgx layout [P, M, POS, H]
                    pgx = pgx_tiles[qb]
                    A = app.tile([P, M, H], f16, tag="A")
                    gb = (g_rows[:, qb, :]
                          .rearrange("p (pp h) -> p pp h", h=H)
                          [:, None, :, :].broadcast_to([P, M, POS, H]))
                    nc.vector.tensor_tensor(out=pgx[:], in0=pgx[:], in1=gb,
                                            op=mult)
                    nc.vector.tensor_tensor(out=pgx[:, :, 0:3, :],
                                            in0=pgx[:, :, 0:3, :],
                                            in1=pgx[:, :, 3:6, :], op=add)
                    nc.vector.tensor_tensor(out=pgx[:, :, 0:1, :],
                                            in0=pgx[:, :, 0:1, :],
                                            in1=pgx[:, :, 1:2, :], op=add)
                    nc.vector.tensor_tensor(out=A[:], in0=pgx[:, :, 0, :],
                                            in1=pgx[:, :, 2, :], op=add)

                    # per m-half: t1 scores, exp (fixed -4 bias keeps f16 E*V
                    # in range; softmax shift-invariant, 1/Z deferred), V
                    # scaling, and PE identity-accumulation — pipelined so PE
                    # aggregation of half 0 overlaps DVE scoring of half 1.
                    # The previous qblock's epilogue is emitted between the
                    # halves so its PSUM reads never stall this one's DVE.
                    E = smallp.tile([P, M, H], f16, tag="E")
                    psA = psump.tile([P, C], f32, tag="agg")
                    psZ = psump.tile([P, H], f32, tag="aggz")
                    emit_score(qb, 0, kvh2[0], A, E)
                    emit_score(qb, 1, kvh2[1], A, E)
                    emit_value(qb, 0, kvh2[0], E, psA, psZ)
                    if pend:
                        emit_epilogue(*pend.pop())
                    emit_value(qb, 1, kvh2[1], E, psA, psZ)
                    pend.append((qb, psA, psZ))
                emit_epilogue(*pend.pop())

    nc.compile()
    return nc


def _wrap_idx(lst):
    """int16 list -> [128, len/16] wrapped (pos i -> [i%16, i//16]) and
    replicated across the 8 groups of 16 partitions."""
    n = lst.shape[0]
    w = np.empty((P, n // 16), np.int16)
    blk = lst.reshape(n // 16, 16).T  # [16, n/16]
    for g in range(8):
        w[g * 16:(g + 1) * 16, :] = blk
    return w


def make_core_inputs(pairwise_g, coset_functions, nbhd_idx,
                     W_q, b_q, W_k, W_l, v, W_in, b_in, W_out, b_out):
    pairwise_g = np.asarray(pairwise_g)
    coset_functions = np.asarray(coset_functions)
    nbhd_idx = np.asarray(nbhd_idx).astype(np.int64)
    W_q = np.asarray(W_q, np.float32)
    b_q = np.asarray(b_q, np.float32)
    W_k = np.asarray(W_k, np.float32)
    W_l = np.asarray(W_l, np.float32)
    v = np.asarray(v, np.float32)
    W_in = np.asarray(W_in, np.float32)
    b_in = np.asarray(b_in, np.float32)
    W_out = np.asarray(W_out, np.float32)
    b_out = np.asarray(b_out, np.float32)

    wq16 = np.ascontiguousarray(
        W_q[CHMAP].T.reshape(2, P, C).astype(np.float16))
    wkv16 = np.ascontiguousarray(
        np.concatenate([W_k[CHMAP].T, W_in[CHMAP].T], axis=1)
        .reshape(2, P, CH2).astype(np.float16))
    wo16 = np.ascontiguousarray(
        W_out.T[CHMAP].reshape(2, P, C).astype(np.float16))
    wl_full = np.zeros((C, H * POS), np.float32)
    for h in range(H):
        for p_ in range(POS):
            wl_full[h * D:(h + 1) * D, p_ * H + h] = W_l[h * D:(h + 1) * D, p_]
    wl16 = np.ascontiguousarray(
        wl_full[CHMAP].reshape(2, P, H * POS).astype(np.float16))
    bqv32 = np.ascontiguousarray(
        (b_q + v.reshape(C))[CHMAP].reshape(2, P, 1).astype(np.float32))
    bqmat = np.ascontiguousarray(
        np.broadcast_to(b_q[CHMAP], (P, C)).astype(np.float32))
    bomat = np.ascontiguousarray(
        np.broadcast_to(W_out @ b_in + b_out, (P, C)).astype(np.float32))
    ident16 = np.eye(P, dtype=np.float16)

    in_maps = []
    for core in range(NCORES):
        b = core // 4
        qs = (core % 4) * NQ
        # roll keys so this core's queries are columns 0..NQ-1 of cosT
        # (the compiled program slices cosT[:, :, 0:NQ] for Q projections);
        # gather idx values are rotated to match the rolled KV table.
        cosT = np.ascontiguousarray(
            np.roll(coset_functions[b], -qs, axis=0)
            .T.reshape(2, P, N).astype(np.float16))
        idx = nbhd_idx[b, qs:qs + NQ]  # [NQ, M]
        idx_rot = (idx - qs) % N

        idxw = np.empty((P, QB, M * P // 16), np.int16)
        for qb in range(QB):
            blk = idx_rot[qb * P:(qb + 1) * P]  # [P(n), M]
            lst = blk.T.reshape(M * P)  # m-major: pos i = m*128 + n
            idxw[:, qb, :] = _wrap_idx(lst.astype(np.int16))

        # host-gathered pairwise_g neighbor rows, h-expanded, f16
        pg_rows = pairwise_g[b, qs + np.arange(NQ)[:, None], idx]  # [NQ,M,POS]
        pgx = np.broadcast_to(pg_rows[:, :, :, None].astype(np.float16),
                              (NQ, M, POS, H))
        pgx = np.ascontiguousarray(pgx).reshape(QB, P, M * POS * H)

        in_maps.append({
            "cosT": cosT,
            "wkv": wkv16, "wq": wq16, "wl": wl16, "wo": wo16,
            "bqv": bqv32, "bqmat": bqmat, "bomat": bomat,
            "ident": ident16, "idxw": idxw, "pgx": pgx,
        })
    return in_maps


def assemble_output(results):
    out = np.empty((B, N, C), np.float32)
    for core in range(NCORES):
        b = core // 4
        qs = (core % 4) * NQ
        o = results[core]["out"]  # [QB, P, C]
        out[b, qs:qs + NQ] = o.reshape(NQ, C)
    return out


def kernel(pairwise_g, coset_functions, mask, nbhd_idx,
           W_q, b_q, W_k, b_k, W_l, b_l, u, v,
           W_in, b_in, W_out, b_out, **_unused):
    from concourse.bass_utils import run_bass_kernel_spmd

    if "nc" not in _compiled:
        _compiled["nc"] = build_bass()
    nc = _compiled["nc"]

    in_maps = make_core_inputs(pairwise_g, coset_functions, nbhd_idx,
                               W_q, b_q, W_k, W_l, v, W_in, b_in,
                               W_out, b_out)
    res = run_bass_kernel_spmd(nc, in_maps, core_ids=list(range(NCORES)))
    return assemble_output(res.results)


# revision 72
# speedup vs baseline: 1.0811x; 1.0019x over previous
"""Trainium2 Bass kernel for nn_EquivairantMultiheadAttention (sparse attention).

Contract: kernel(**inputs) takes the FULL unsharded numpy inputs (as produced by
setup_inputs()) and returns the FULL (B, N, COUT) float32 output.

Sharding: 8 cores = data-parallel over batch (2) x sequence-parallel over the
query dim n (4 slices of 512).

Architecture (v1, f16):
 - All PE inputs and gathered tables in float16; channel order permuted to
   d-major (ch' = d*8 + h) so per-head broadcasts have contiguous last dims
   (enables the DVE 2x 16-bit mode on every big elementwise op).
 - K and V rows stored interleaved in one DRAM table (1KB rows); one
   dma_gather per 1024 neighbors fetches both (half the descriptors).
 - pairwise_g neighbor rows are gathered on HOST (pure data selection) and
   streamed as a dense (512, 64, 6, 8) f16 tensor (h-expanded for 2x mode).
 - Scores: t1 = kg*Q elementwise + in-place halving tree over d (all f16 2x);
   t2 = pgx*G + tree; t3/b_k/b_l terms are constant over the softmax axis m
   and drop out. Softmax exp on the Act engine; normalization deferred.
 - Aggregation: per-m matmuls with identity lhsT accumulate E-scaled V rows
   in PSUM (replaces a DVE reduction tree); 1/Z applied on PSUM evac;
 - b_in folded into b_out on host: out = W_out@agg + (W_out@b_in + b_out).
"""

import math
import sys

import numpy as np

sys.path.insert(0, "/opt/trn_rl_repo")

B, N, M = 2, 2048, 64
C = 256  # CIN == COUT
H, D, POS = 8, 32, 6
NQ = 512  # queries per core
QB = 4  # query blocks of 128 per core
P = 128
NCORES = 8
INV_SQRT_D = 1.0 / math.sqrt(D)
NIC = 1024  # max idxs per dma_gather call (HW: larger crashes the exec unit)
CH2 = 2 * C  # combined K|V row width

_compiled = {}

# channel permutation: ch' = d*8 + h  <->  ch = h*32 + d
CHMAP = np.array([(cp % 8) * D + (cp // 8) for cp in range(C)], dtype=np.int64)


def build_bass():
    import concourse.bacc as bacc
    import concourse.mybir as mybir
    import concourse.tile as tile

    dt = mybir.dt
    nc = bacc.Bacc("TRN2", target_bir_lowering=False, debug=False,
                   enable_asserts=False, num_devices=NCORES)

    f16 = dt.float16
    f32 = dt.float32
    i16 = dt.int16

    # ---- DRAM inputs (per core) ----
    # cosT is rolled per-core so this core's NQ query columns are 0..NQ-1
    # (gather idx values are rotated to match on host).
    d_cosT = nc.dram_tensor("cosT", [2, P, N], f16, kind="ExternalInput")
    d_wkv = nc.dram_tensor("wkv", [2, P, CH2], f16, kind="ExternalInput")
    d_wq = nc.dram_tensor("wq", [2, P, C], f16, kind="ExternalInput")
    d_wl = nc.dram_tensor("wl", [2, P, H * POS], f16, kind="ExternalInput")
    d_wo = nc.dram_tensor("wo", [2, P, C], f16, kind="ExternalInput")
    d_bqv = nc.dram_tensor("bqv", [2, P, 1], f32, kind="ExternalInput")
    d_bqmat = nc.dram_tensor("bqmat", [P, C], f32, kind="ExternalInput")
    d_bomat = nc.dram_tensor("bomat", [P, C], f32, kind="ExternalInput")
    d_ident = nc.dram_tensor("ident", [P, P], f16, kind="ExternalInput")
    d_idxw = nc.dram_tensor("idxw", [P, QB, M * P // 16], i16,
                            kind="ExternalInput")
    d_pgx = nc.dram_tensor("pgx", [QB, P, M * POS * H], f16,
                           kind="ExternalInput")
    d_out = nc.dram_tensor("out", [QB, P, C], f32, kind="ExternalOutput")

    add = mybir.AluOpType.add
    mult = mybir.AluOpType.mult

    with tile.TileContext(nc) as tc:
        with (
            tc.tile_pool(name="const", bufs=1) as constp,
            tc.tile_pool(name="dram", bufs=1, space="DRAM") as dramp,
            tc.tile_pool(name="psum", bufs=2, space="PSUM") as psump,
            tc.tile_pool(name="evac", bufs=2) as evacp,
        ):
            # critical-path loads first: cosT gates the KV table -> gathers
            cosT = constp.tile([P, 2, N], f16)
            wkv = constp.tile([P, 2, CH2], f16)
            ident = constp.tile([P, P], f16)
            for cc in range(2):
                nc.sync.dma_start(cosT[:, cc, :], d_cosT[cc])
                nc.sync.dma_start(wkv[:, cc, :], d_wkv[cc])
            nc.sync.dma_start(ident[:], d_ident.ap())
            wq = constp.tile([P, 2, C], f16)
            wl = constp.tile([P, 2, H * POS], f16)
            wo = constp.tile([P, 2, C], f16)
            for cc in range(2):
                nc.sync.dma_start(wq[:, cc, :], d_wq[cc])
                nc.sync.dma_start(wl[:, cc, :], d_wl[cc])
                nc.sync.dma_start(wo[:, cc, :], d_wo[cc])
            bqv = constp.tile([P, 2, 1], f32)
            nc.sync.dma_start(bqv[:], d_bqv.ap().rearrange("c p one -> p c one"))
            bqmat = constp.tile([P, C], f32)
            bomat = constp.tile([P, C], f32)
            nc.sync.dma_start(bqmat[:], d_bqmat.ap())
            nc.sync.dma_start(bomat[:], d_bomat.ap())
            idxw = constp.tile([P, QB, M * P // 16], i16)
            nc.sync.dma_start(idxw[:], d_idxw.ap())
            nbias = constp.tile([P, 1], f32)
            nc.vector.memset(nbias[:], -4.0)

            # combined K|V rows (f16, permuted channels) in DRAM scratch
            kvdr = dramp.tile([N, CH2], f16)

            q_rows = constp.tile([P, QB, C], f16)
            g_rows = constp.tile([P, QB, H * POS], f16)

            with tc.tile_pool(name="prep", bufs=1) as prepp:
                # tiny matmuls advance the PE pstate ramp almost for free so
                # the KV-table matmuls run at speed
                psW = psump.tile([P, 2], f32, tag="pst", bufs=1)
                for i in range(12):
                    nc.tensor.matmul(psW[:], lhsT=ident[:], rhs=ident[:, 0:2],
                                     start=(i == 0), stop=(i == 11))
                JB = 8  # jt rows per batched kvdr write
                for jg in range(N // P // JB):
                    kv_sb = evacp.tile([P, JB, CH2], f16, tag="kvevac")
                    for j in range(JB):
                        jt = jg * JB + j
                        ps = psump.tile([P, CH2], f32, tag="ps", bufs=3)
                        for cc in range(2):
                            nc.tensor.matmul(
                                ps[:],
                                lhsT=cosT[:, cc, jt * P:(jt + 1) * P],
                                rhs=wkv[:, cc, :],
                                start=(cc == 0), stop=(cc == 1))
                        nc.vector.tensor_copy(kv_sb[:, j, :], ps[:])
                    nc.sync.dma_start(
                        kvdr[jg * JB * P:(jg + 1) * JB * P, :]
                        .rearrange("(j p) w -> p j w", j=JB),
                        kv_sb[:])

                for nt in range(QB):
                    ps = psump.tile([P, C], f32, tag="agg", bufs=2)
                    for cc in range(2):
                        nc.tensor.matmul(ps[:],
                                         lhsT=cosT[:, cc, nt * P:(nt + 1) * P],
                                         rhs=wq[:, cc, :],
                                         start=(cc == 0), stop=(cc == 1))
                    nc.vector.tensor_tensor(out=q_rows[:, nt, :], in0=ps[:],
                                            in1=bqmat[:], op=add)

                qvT = prepp.tile([P, 2, NQ], f16)
                for cc2 in range(2):
                    ps = psump.tile([P, NQ], f32, tag="agg", bufs=2)
                    for cc in range(2):
                        nc.tensor.matmul(ps[:],
                                         lhsT=wq[:, cc, cc2 * P:(cc2 + 1) * P],
                                         rhs=cosT[:, cc, 0:NQ],
                                         start=(cc == 0), stop=(cc == 1))
                    nc.vector.tensor_tensor(
                        out=qvT[:, cc2, :], in0=ps[:],
                        in1=bqv[:, cc2, :].broadcast_to([P, NQ]), op=add)
                for nt in range(QB):
                    ps = psump.tile([P, H * POS], f32, tag="agg", bufs=2)
                    for cc in range(2):
                        nc.tensor.matmul(ps[:],
                                         lhsT=qvT[:, cc, nt * P:(nt + 1) * P],
                                         rhs=wl[:, cc, :],
                                         start=(cc == 0), stop=(cc == 1))
                    nc.scalar.copy(g_rows[:, nt, :], ps[:])

            # ---- main loop: per query block ----
            with (
                tc.tile_pool(name="kvp", bufs=4) as kvp,
                tc.tile_pool(name="pgp", bufs=4) as pgp,
                tc.tile_pool(name="ap", bufs=4) as app,
                tc.tile_pool(name="small", bufs=2) as smallp,
            ):
                MH = M // 2

                def emit_score(qb, mh, kvh, A, E):
                    # chunked at gather-call granularity (8 m per call) so
                    # scoring starts as soon as the first call lands
                    ms = slice(mh * MH, (mh + 1) * MH)
                    CHK = 16
                    for ck in range(MH // CHK):
                        cs0 = mh * MH + ck * CHK
                        kg = (kvh[:, ck * CHK:(ck + 1) * CHK, 0:C]
                              .rearrange("p m (d h) -> p m d h", h=H))
                        nc.vector.tensor_tensor(
                            out=kg, in0=kg,
                            in1=q_rows[:, qb, :][:, None, :]
                                .broadcast_to([P, CHK, C])
                                .rearrange("p m (d h) -> p m d h", h=H),
                            op=mult)
                        dd = D // 2
                        while dd >= 1:
                            nc.vector.tensor_tensor(out=kg[:, :, 0:dd, :],
                                                    in0=kg[:, :, 0:dd, :],
                                                    in1=kg[:, :, dd:2 * dd, :],
                                                    op=add)
                            dd //= 2
                        nc.vector.tensor_tensor(
                            out=A[:, cs0:cs0 + CHK, :],
                            in0=A[:, cs0:cs0 + CHK, :],
                            in1=kg[:, :, 0, :], op=add)
                    nc.scalar.activation(
                        out=E[:, ms, :], in_=A[:, ms, :],
                        func=mybir.ActivationFunctionType.Exp,
                        scale=INV_SQRT_D, bias=nbias[:])

                def emit_value(qb, mh, kvh, E, psA, psZ):
                    # Z via tiny identity matmuls: real work that also
                    # advances the PE pstate ramp (they only need E, not vg)
                    for mm in range(MH):
                        m0 = mh * MH + mm
                        nc.tensor.matmul(
                            psZ[:], lhsT=ident[:], rhs=E[:, m0, :],
                            start=(mh == 0 and mm == 0),
                            stop=(mh == 1 and mm == MH - 1),
                            skip_group_check=True)
                    # chunk the E-scaling so the first aggregation matmuls
                    # start ~3us earlier and PE stays continuously busy
                    CHK = 8
                    for ck in range(MH // CHK):
                        cs0 = mh * MH + ck * CHK
                        vgc = (kvh[:, ck * CHK:(ck + 1) * CHK, C:CH2]
                               .rearrange("p m (d h) -> p m d h", h=H))
                        nc.vector.tensor_tensor(
                            out=vgc, in0=vgc,
                            in1=E[:, cs0:cs0 + CHK, None, :]
                                .broadcast_to([P, CHK, D, H]),
                            op=mult)
                        for mm in range(ck * CHK, (ck + 1) * CHK):
                            nc.tensor.matmul(
                                psA[:], lhsT=ident[:],
                                rhs=kvh[:, mm, C:CH2],
                                start=(mh == 0 and mm == 0),
                                stop=(mh == 1 and mm == MH - 1),
                                skip_group_check=True)

                def emit_epilogue(qb, psA, psZ):
                    # fold psum halves, normalize (1/Z); transpose + out proj
                    rz = smallp.tile([P, H], f32, tag="rz", name=f"rz{qb}")
                    nc.vector.reciprocal(rz[:], psZ[:])
                    aggN = smallp.tile([P, C], f16, tag="aggN",
                                       name=f"aggN{qb}")
                    nc.vector.tensor_tensor(
                        out=aggN[:].rearrange("p (d h) -> p d h", h=H),
                        in0=psA[:].rearrange("p (d h) -> p d h", h=H),
                        in1=rz[:][:, None, :].broadcast_to([P, D, H]),
                        op=mult)
                    aggT = smallp.tile([P, 2, P], f16, tag="aggT",
                                       name=f"aggT{qb}")
                    for cc in range(2):
                        pst = psump.tile([P, P], f16, tag="pst", bufs=1,
                                         name=f"pst{qb}_{cc}")
                        nc.tensor.transpose(pst[:],
                                            aggN[:, cc * P:(cc + 1) * P],
                                            ident[:])
                        nc.scalar.copy(aggT[:, cc, :], pst[:])
                    psO = psump.tile([P, C], f32, tag="pst", bufs=1,
                                     name=f"psO{qb}")
                    for cc in range(2):
                        nc.tensor.matmul(psO[:], lhsT=aggT[:, cc, :],
                                         rhs=wo[:, cc, :],
                                         start=(cc == 0), stop=(cc == 1))
                    out_sb = smallp.tile([P, C], f32, tag="outsb",
                                         name=f"outsb{qb}")
                    nc.vector.tensor_tensor(out=out_sb[:], in0=psO[:],
                                            in1=bomat[:], op=add)
                    nc.sync.dma_start(d_out[qb], out_sb[:])

                pgx_tiles = {}
                pend = []  # deferred epilogue args (software pipelining)
                for qb in range(QB):
                    pgx_tiles[qb] = pgp.tile([P, M, POS, H], f16, tag="pgx",
                                             name=f"pgx{qb}")
                    nc.sync.dma_start(pgx_tiles[qb][:], d_pgx[qb])
                for qb in range(QB):
                    kvh2 = []
                    for mh in range(2):
                        kvh = kvp.tile([P, MH, CH2], f16, tag="kv",
                                       name=f"kv{qb}_{mh}")
                        kvh2.append(kvh)
                        for kk in range(4):
                            k = mh * 4 + kk
                            nc.gpsimd.dma_gather(
                                out_ap=kvh[:, kk * (NIC // P):
                                           (kk + 1) * (NIC // P), :],
                                in_ap=kvdr[:],
                                idxs_ap=idxw[:, qb, k * (NIC // 16):
                                             (k + 1) * (NIC // 16)],
                                num_idxs=NIC, num_idxs_reg=NIC,
                                elem_size=CH2)

                    # t2 -> A (rel-pos term); pgx layout [P, M, POS, H]
                    pgx = pgx_tiles[qb]
                    A = app.tile([P, M, H], f16, tag="A")
                    gb = (g_rows[:, qb, :]
                          .rearrange("p (pp h) -> p pp h", h=H)
                          [:, None, :, :].broadcast_to([P, M, POS, H]))
                    nc.vector.tensor_tensor(out=pgx[:], in0=pgx[:], in1=gb,
                                            op=mult)
                    nc.vector.tensor_tensor(out=pgx[:, :, 0:3, :],
                                            in0=pgx[:, :, 0:3, :],
                                            in1=pgx[:, :, 3:6, :], op=add)
                    nc.vector.tensor_tensor(out=pgx[:, :, 0:1, :],
                                            in0=pgx[:, :, 0:1, :],
                                            in1=pgx[:, :, 1:2, :], op=add)
                    nc.vector.tensor_tensor(out=A[:], in0=pgx[:, :, 0, :],
                                            in1=pgx[:, :, 2, :], op=add)

                    # per m-half: t1 scores, exp (fixed -4 bias keeps f16 E*V
                    # in range; softmax shift-invariant, 1/Z deferred), V
                    # scaling, and PE identity-accumulation — pipelined so PE
                    # aggregation of half 0 overlaps DVE scoring of half 1.
                    # The previous qblock's epilogue is emitted between the
                    # halves so its PSUM reads never stall this one's DVE.
                    E = smallp.tile([P, M, H], f16, tag="E")
                    psA = psump.tile([P, C], f32, tag="agg")
                    psZ = psump.tile([P, H], f32, tag="aggz")
                    emit_score(qb, 0, kvh2[0], A, E)
                    emit_score(qb, 1, kvh2[1], A, E)
                    emit_value(qb, 0, kvh2[0], E, psA, psZ)
                    if pend:
                        emit_epilogue(*pend.pop())
                    emit_value(qb, 1, kvh2[1], E, psA, psZ)
                    pend.append((qb, psA, psZ))
                emit_epilogue(*pend.pop())

    nc.compile()
    return nc


def _wrap_idx(lst):
    """int16 list -> [128, len/16] wrapped (pos i -> [i%16, i//16]) and
    replicated across the 8 groups of 16 partitions."""
    n = lst.shape[0]
    w = np.empty((P, n // 16), np.int16)
    blk = lst.reshape(n // 16, 16).T  # [16, n/16]
    for g in range(8):
        w[g * 16:(g + 1) * 16, :] = blk
    return w


def make_core_inputs(pairwise_g, coset_functions, nbhd_idx,
                     W_q, b_q, W_k, W_l, v, W_in, b_in, W_out, b_out):
    pairwise_g = np.asarray(pairwise_g)
    coset_functions = np.asarray(coset_functions)
    nbhd_idx = np.asarray(nbhd_idx).astype(np.int64)
    W_q = np.asarray(W_q, np.float32)
    b_q = np.asarray(b_q, np.float32)
    W_k = np.asarray(W_k, np.float32)
    W_l = np.asarray(W_l, np.float32)
    v = np.asarray(v, np.float32)
    W_in = np.asarray(W_in, np.float32)
    b_in = np.asarray(b_in, np.float32)
    W_out = np.asarray(W_out, np.float32)
    b_out = np.asarray(b_out, np.float32)

    wq16 = np.ascontiguousarray(
        W_q[CHMAP].T.reshape(2, P, C).astype(np.float16))
    wkv16 = np.ascontiguousarray(
        np.concatenate([W_k[CHMAP].T, W_in[CHMAP].T], axis=1)
        .reshape(2, P, CH2).astype(np.float16))
    wo16 = np.ascontiguousarray(
        W_out.T[CHMAP].reshape(2, P, C).astype(np.float16))
    wl_full = np.zeros((C, H * POS), np.float32)
    for h in range(H):
        for p_ in range(POS):
            wl_full[h * D:(h + 1) * D, p_ * H + h] = W_l[h * D:(h + 1) * D, p_]
    wl16 = np.ascontiguousarray(
        wl_full[CHMAP].reshape(2, P, H * POS).astype(np.float16))
    bqv32 = np.ascontiguousarray(
        (b_q + v.reshape(C))[CHMAP].reshape(2, P, 1).astype(np.float32))
    bqmat = np.ascontiguousarray(
        np.broadcast_to(b_q[CHMAP], (P, C)).astype(np.float32))
    bomat = np.ascontiguousarray(
        np.broadcast_to(W_out @ b_in + b_out, (P, C)).astype(np.float32))
    ident16 = np.eye(P, dtype=np.float16)

    in_maps = []
    for core in range(NCORES):
        b = core // 4
        qs = (core % 4) * NQ
        # roll keys so this core's queries are columns 0..NQ-1 of cosT
        # (the compiled program slices cosT[:, :, 0:NQ] for Q projections);
        # gather idx values are rotated to match the rolled KV table.
        cosT = np.ascontiguousarray(
            np.roll(coset_functions[b], -qs, axis=0)
            .T.reshape(2, P, N).astype(np.float16))
        idx = nbhd_idx[b, qs:qs + NQ]  # [NQ, M]
        idx_rot = (idx - qs) % N

        idxw = np.empty((P, QB, M * P // 16), np.int16)
        for qb in range(QB):
            blk = idx_rot[qb * P:(qb + 1) * P]  # [P(n), M]
            lst = blk.T.reshape(M * P)  # m-major: pos i = m*128 + n
            idxw[:, qb, :] = _wrap_idx(lst.astype(np.int16))

        # host-gathered pairwise_g neighbor rows, h-expanded, f16
        pg_rows = pairwise_g[b, qs + np.arange(NQ)[:, None], idx]  # [NQ,M,POS]
        pgx = np.broadcast_to(pg_rows[:, :, :, None].astype(np.float16),
                              (NQ, M, POS, H))
        pgx = np.ascontiguousarray(pgx).reshape(QB, P, M * POS * H)

        in_maps.append({
            "cosT": cosT,
            "wkv": wkv16, "wq": wq16, "wl": wl16, "wo": wo16,
            "bqv": bqv32, "bqmat": bqmat, "bomat": bomat,
            "ident": ident16, "idxw": idxw, "pgx": pgx,
        })
    return in_maps


def assemble_output(results):
    out = np.empty((B, N, C), np.float32)
    for core in range(NCORES):
        b = core // 4
        qs = (core % 4) * NQ
        o = results[core]["out"]  # [QB, P, C]
        out[b, qs:qs + NQ] = o.reshape(NQ, C)
    return out


def kernel(pairwise_g, coset_functions, mask, nbhd_idx,
           W_q, b_q, W_k, b_k, W_l, b_l, u, v,
           W_in, b_in, W_out, b_out, **_unused):
    from concourse.bass_utils import run_bass_kernel_spmd

    if "nc" not in _compiled:
        _compiled["nc"] = build_bass()
    nc = _compiled["nc"]

    in_maps = make_core_inputs(pairwise_g, coset_functions, nbhd_idx,
                               W_q, b_q, W_k, W_l, v, W_in, b_in,
                               W_out, b_out)
    res = run_bass_kernel_spmd(nc, in_maps, core_ids=list(range(NCORES)))
    return assemble_output(res.results)


# revision 73
# speedup vs baseline: 1.0864x; 1.0050x over previous
"""Trainium2 Bass kernel for nn_EquivairantMultiheadAttention (sparse attention).

Contract: kernel(**inputs) takes the FULL unsharded numpy inputs (as produced by
setup_inputs()) and returns the FULL (B, N, COUT) float32 output.

Sharding: 8 cores = data-parallel over batch (2) x sequence-parallel over the
query dim n (4 slices of 512).

Architecture (v1, f16):
 - All PE inputs and gathered tables in float16; channel order permuted to
   d-major (ch' = d*8 + h) so per-head broadcasts have contiguous last dims
   (enables the DVE 2x 16-bit mode on every big elementwise op).
 - K and V rows stored interleaved in one DRAM table (1KB rows); one
   dma_gather per 1024 neighbors fetches both (half the descriptors).
 - pairwise_g neighbor rows are gathered on HOST (pure data selection) and
   streamed as a dense (512, 64, 6, 8) f16 tensor (h-expanded for 2x mode).
 - Scores: t1 = kg*Q elementwise + in-place halving tree over d (all f16 2x);
   t2 = pgx*G + tree; t3/b_k/b_l terms are constant over the softmax axis m
   and drop out. Softmax exp on the Act engine; normalization deferred.
 - Aggregation: per-m matmuls with identity lhsT accumulate E-scaled V rows
   in PSUM (replaces a DVE reduction tree); 1/Z applied on PSUM evac;
 - b_in folded into b_out on host: out = W_out@agg + (W_out@b_in + b_out).
"""

import math
import sys

import numpy as np

sys.path.insert(0, "/opt/trn_rl_repo")

B, N, M = 2, 2048, 64
C = 256  # CIN == COUT
H, D, POS = 8, 32, 6
NQ = 512  # queries per core
QB = 4  # query blocks of 128 per core
P = 128
NCORES = 8
INV_SQRT_D = 1.0 / math.sqrt(D)
NIC = 1024  # max idxs per dma_gather call (HW: larger crashes the exec unit)
CH2 = 2 * C  # combined K|V row width

_compiled = {}

# channel permutation: ch' = d*8 + h  <->  ch = h*32 + d
CHMAP = np.array([(cp % 8) * D + (cp // 8) for cp in range(C)], dtype=np.int64)


def build_bass():
    import concourse.bacc as bacc
    import concourse.mybir as mybir
    import concourse.tile as tile

    dt = mybir.dt
    nc = bacc.Bacc("TRN2", target_bir_lowering=False, debug=False,
                   enable_asserts=False, num_devices=NCORES)

    f16 = dt.float16
    f32 = dt.float32
    i16 = dt.int16

    # ---- DRAM inputs (per core) ----
    # cosT is rolled per-core so this core's NQ query columns are 0..NQ-1
    # (gather idx values are rotated to match on host).
    d_cosT = nc.dram_tensor("cosT", [2, P, N], f16, kind="ExternalInput")
    d_wkv = nc.dram_tensor("wkv", [2, P, CH2], f16, kind="ExternalInput")
    d_wq = nc.dram_tensor("wq", [2, P, C], f16, kind="ExternalInput")
    d_wl = nc.dram_tensor("wl", [2, P, H * POS], f16, kind="ExternalInput")
    d_wo = nc.dram_tensor("wo", [2, P, C], f16, kind="ExternalInput")
    d_bqv = nc.dram_tensor("bqv", [2, P, 1], f32, kind="ExternalInput")
    d_bqmat = nc.dram_tensor("bqmat", [P, C], f32, kind="ExternalInput")
    d_bomat = nc.dram_tensor("bomat", [P, C], f32, kind="ExternalInput")
    d_ident = nc.dram_tensor("ident", [P, P], f16, kind="ExternalInput")
    d_idxw = nc.dram_tensor("idxw", [P, QB, M * P // 16], i16,
                            kind="ExternalInput")
    d_pgx = nc.dram_tensor("pgx", [QB, P, M * POS * H], f16,
                           kind="ExternalInput")
    d_out = nc.dram_tensor("out", [QB, P, C], f32, kind="ExternalOutput")

    add = mybir.AluOpType.add
    mult = mybir.AluOpType.mult

    with tile.TileContext(nc) as tc:
        with (
            tc.tile_pool(name="const", bufs=1) as constp,
            tc.tile_pool(name="dram", bufs=1, space="DRAM") as dramp,
            tc.tile_pool(name="psum", bufs=2, space="PSUM") as psump,
            tc.tile_pool(name="evac", bufs=2) as evacp,
        ):
            # critical-path loads first: cosT gates the KV table -> gathers
            cosT = constp.tile([P, 2, N], f16)
            wkv = constp.tile([P, 2, CH2], f16)
            ident = constp.tile([P, P], f16)
            for cc in range(2):
                nc.sync.dma_start(cosT[:, cc, :], d_cosT[cc])
                nc.sync.dma_start(wkv[:, cc, :], d_wkv[cc])
            nc.sync.dma_start(ident[:], d_ident.ap())
            wq = constp.tile([P, 2, C], f16)
            wl = constp.tile([P, 2, H * POS], f16)
            wo = constp.tile([P, 2, C], f16)
            for cc in range(2):
                nc.sync.dma_start(wq[:, cc, :], d_wq[cc])
                nc.sync.dma_start(wl[:, cc, :], d_wl[cc])
                nc.sync.dma_start(wo[:, cc, :], d_wo[cc])
            bqv = constp.tile([P, 2, 1], f32)
            nc.sync.dma_start(bqv[:], d_bqv.ap().rearrange("c p one -> p c one"))
            bqmat = constp.tile([P, C], f32)
            bomat = constp.tile([P, C], f32)
            nc.sync.dma_start(bqmat[:], d_bqmat.ap())
            nc.sync.dma_start(bomat[:], d_bomat.ap())
            idxw = constp.tile([P, QB, M * P // 16], i16)
            nc.sync.dma_start(idxw[:], d_idxw.ap())
            nbias = constp.tile([P, 1], f32)
            nc.vector.memset(nbias[:], -4.0)

            # combined K|V rows (f16, permuted channels) in DRAM scratch
            kvdr = dramp.tile([N, CH2], f16)

            q_rows = constp.tile([P, QB, C], f16)
            g_rows = constp.tile([P, QB, H * POS], f16)

            with tc.tile_pool(name="prep", bufs=1) as prepp:
                # tiny matmuls advance the PE pstate ramp almost for free so
                # the KV-table matmuls run at speed
                psW = psump.tile([P, 2], f32, tag="pst", bufs=1)
                for i in range(12):
                    nc.tensor.matmul(psW[:], lhsT=ident[:], rhs=ident[:, 0:2],
                                     start=(i == 0), stop=(i == 11))
                JB = 8  # jt rows per batched kvdr write
                for jg in range(N // P // JB):
                    kv_sb = evacp.tile([P, JB, CH2], f16, tag="kvevac")
                    for j in range(JB):
                        jt = jg * JB + j
                        ps = psump.tile([P, CH2], f32, tag="ps", bufs=3)
                        for cc in range(2):
                            nc.tensor.matmul(
                                ps[:],
                                lhsT=cosT[:, cc, jt * P:(jt + 1) * P],
                                rhs=wkv[:, cc, :],
                                start=(cc == 0), stop=(cc == 1))
                        nc.vector.tensor_copy(kv_sb[:, j, :], ps[:])
                    nc.sync.dma_start(
                        kvdr[jg * JB * P:(jg + 1) * JB * P, :]
                        .rearrange("(j p) w -> p j w", j=JB),
                        kv_sb[:])

                for nt in range(QB):
                    ps = psump.tile([P, C], f32, tag="agg", bufs=2)
                    for cc in range(2):
                        nc.tensor.matmul(ps[:],
                                         lhsT=cosT[:, cc, nt * P:(nt + 1) * P],
                                         rhs=wq[:, cc, :],
                                         start=(cc == 0), stop=(cc == 1))
                    nc.vector.tensor_tensor(out=q_rows[:, nt, :], in0=ps[:],
                                            in1=bqmat[:], op=add)

                qvT = prepp.tile([P, 2, NQ], f16)
                for cc2 in range(2):
                    ps = psump.tile([P, NQ], f32, tag="agg", bufs=2)
                    for cc in range(2):
                        nc.tensor.matmul(ps[:],
                                         lhsT=wq[:, cc, cc2 * P:(cc2 + 1) * P],
                                         rhs=cosT[:, cc, 0:NQ],
                                         start=(cc == 0), stop=(cc == 1))
                    nc.vector.tensor_tensor(
                        out=qvT[:, cc2, :], in0=ps[:],
                        in1=bqv[:, cc2, :].broadcast_to([P, NQ]), op=add)
                for nt in range(QB):
                    ps = psump.tile([P, H * POS], f32, tag="agg", bufs=2)
                    for cc in range(2):
                        nc.tensor.matmul(ps[:],
                                         lhsT=qvT[:, cc, nt * P:(nt + 1) * P],
                                         rhs=wl[:, cc, :],
                                         start=(cc == 0), stop=(cc == 1))
                    nc.scalar.copy(g_rows[:, nt, :], ps[:])

            # ---- main loop: per query block ----
            with (
                tc.tile_pool(name="kvp", bufs=4) as kvp,
                tc.tile_pool(name="pgp", bufs=4) as pgp,
                tc.tile_pool(name="ap", bufs=4) as app,
                tc.tile_pool(name="small", bufs=2) as smallp,
            ):
                MH = M // 2

                def emit_score(qb, mh, kvh, A, E):
                    # chunked at gather-call granularity (8 m per call) so
                    # scoring starts as soon as the first call lands
                    ms = slice(mh * MH, (mh + 1) * MH)
                    CHK = 16
                    for ck in range(MH // CHK):
                        cs0 = mh * MH + ck * CHK
                        kg = (kvh[:, ck * CHK:(ck + 1) * CHK, 0:C]
                              .rearrange("p m (d h) -> p m d h", h=H))
                        nc.vector.tensor_tensor(
                            out=kg, in0=kg,
                            in1=q_rows[:, qb, :][:, None, :]
                                .broadcast_to([P, CHK, C])
                                .rearrange("p m (d h) -> p m d h", h=H),
                            op=mult)
                        dd = D // 2
                        while dd >= 1:
                            nc.vector.tensor_tensor(out=kg[:, :, 0:dd, :],
                                                    in0=kg[:, :, 0:dd, :],
                                                    in1=kg[:, :, dd:2 * dd, :],
                                                    op=add)
                            dd //= 2
                        nc.vector.tensor_tensor(
                            out=A[:, cs0:cs0 + CHK, :],
                            in0=A[:, cs0:cs0 + CHK, :],
                            in1=kg[:, :, 0, :], op=add)
                        nc.scalar.activation(
                            out=E[:, cs0:cs0 + CHK, :],
                            in_=A[:, cs0:cs0 + CHK, :],
                            func=mybir.ActivationFunctionType.Exp,
                            scale=INV_SQRT_D, bias=nbias[:])

                def emit_value(qb, mh, kvh, E, psA, psZ):
                    # Z via tiny identity matmuls: real work that also
                    # advances the PE pstate ramp (they only need E, not vg)
                    for mm in range(MH):
                        m0 = mh * MH + mm
                        nc.tensor.matmul(
                            psZ[:], lhsT=ident[:], rhs=E[:, m0, :],
                            start=(mh == 0 and mm == 0),
                            stop=(mh == 1 and mm == MH - 1),
                            skip_group_check=True)
                    # chunk the E-scaling so the first aggregation matmuls
                    # start ~3us earlier and PE stays continuously busy
                    CHK = 8
                    for ck in range(MH // CHK):
                        cs0 = mh * MH + ck * CHK
                        vgc = (kvh[:, ck * CHK:(ck + 1) * CHK, C:CH2]
                               .rearrange("p m (d h) -> p m d h", h=H))
                        nc.vector.tensor_tensor(
                            out=vgc, in0=vgc,
                            in1=E[:, cs0:cs0 + CHK, None, :]
                                .broadcast_to([P, CHK, D, H]),
                            op=mult)
                        for mm in range(ck * CHK, (ck + 1) * CHK):
                            nc.tensor.matmul(
                                psA[:], lhsT=ident[:],
                                rhs=kvh[:, mm, C:CH2],
                                start=(mh == 0 and mm == 0),
                                stop=(mh == 1 and mm == MH - 1),
                                skip_group_check=True)

                def emit_epilogue(qb, psA, psZ):
                    # fold psum halves, normalize (1/Z); transpose + out proj
                    rz = smallp.tile([P, H], f32, tag="rz", name=f"rz{qb}")
                    nc.vector.reciprocal(rz[:], psZ[:])
                    aggN = smallp.tile([P, C], f16, tag="aggN",
                                       name=f"aggN{qb}")
                    nc.vector.tensor_tensor(
                        out=aggN[:].rearrange("p (d h) -> p d h", h=H),
                        in0=psA[:].rearrange("p (d h) -> p d h", h=H),
                        in1=rz[:][:, None, :].broadcast_to([P, D, H]),
                        op=mult)
                    aggT = smallp.tile([P, 2, P], f16, tag="aggT",
                                       name=f"aggT{qb}")
                    for cc in range(2):
                        pst = psump.tile([P, P], f16, tag="pst", bufs=1,
                                         name=f"pst{qb}_{cc}")
                        nc.tensor.transpose(pst[:],
                                            aggN[:, cc * P:(cc + 1) * P],
                                            ident[:])
                        nc.scalar.copy(aggT[:, cc, :], pst[:])
                    psO = psump.tile([P, C], f32, tag="pst", bufs=1,
                                     name=f"psO{qb}")
                    for cc in range(2):
                        nc.tensor.matmul(psO[:], lhsT=aggT[:, cc, :],
                                         rhs=wo[:, cc, :],
                                         start=(cc == 0), stop=(cc == 1))
                    out_sb = smallp.tile([P, C], f32, tag="outsb",
                                         name=f"outsb{qb}")
                    nc.vector.tensor_tensor(out=out_sb[:], in0=psO[:],
                                            in1=bomat[:], op=add)
                    nc.sync.dma_start(d_out[qb], out_sb[:])

                pgx_tiles = {}
                pend = []  # deferred epilogue args (software pipelining)
                for qb in range(QB):
                    pgx_tiles[qb] = pgp.tile([P, M, POS, H], f16, tag="pgx",
                                             name=f"pgx{qb}")
                    nc.sync.dma_start(pgx_tiles[qb][:], d_pgx[qb])
                for qb in range(QB):
                    kvh2 = []
                    for mh in range(2):
                        kvh = kvp.tile([P, MH, CH2], f16, tag="kv",
                                       name=f"kv{qb}_{mh}")
                        kvh2.append(kvh)
                        for kk in range(4):
                            k = mh * 4 + kk
                            nc.gpsimd.dma_gather(
                                out_ap=kvh[:, kk * (NIC // P):
                                           (kk + 1) * (NIC // P), :],
                                in_ap=kvdr[:],
                                idxs_ap=idxw[:, qb, k * (NIC // 16):
                                             (k + 1) * (NIC // 16)],
                                num_idxs=NIC, num_idxs_reg=NIC,
                                elem_size=CH2)

                    # t2 -> A (rel-pos term); pgx layout [P, M, POS, H]
                    pgx = pgx_tiles[qb]
                    A = app.tile([P, M, H], f16, tag="A")
                    gb = (g_rows[:, qb, :]
                          .rearrange("p (pp h) -> p pp h", h=H)
                          [:, None, :, :].broadcast_to([P, M, POS, H]))
                    nc.vector.tensor_tensor(out=pgx[:], in0=pgx[:], in1=gb,
                                            op=mult)
                    nc.vector.tensor_tensor(out=pgx[:, :, 0:3, :],
                                            in0=pgx[:, :, 0:3, :],
                                            in1=pgx[:, :, 3:6, :], op=add)
                    nc.vector.tensor_tensor(out=pgx[:, :, 0:1, :],
                                            in0=pgx[:, :, 0:1, :],
                                            in1=pgx[:, :, 1:2, :], op=add)
                    nc.vector.tensor_tensor(out=A[:], in0=pgx[:, :, 0, :],
                                            in1=pgx[:, :, 2, :], op=add)

                    # per m-half: t1 scores, exp (fixed -4 bias keeps f16 E*V
                    # in range; softmax shift-invariant, 1/Z deferred), V
                    # scaling, and PE identity-accumulation — pipelined so PE
                    # aggregation of half 0 overlaps DVE scoring of half 1.
                    # The previous qblock's epilogue is emitted between the
                    # halves so its PSUM reads never stall this one's DVE.
                    E = smallp.tile([P, M, H], f16, tag="E")
                    psA = psump.tile([P, C], f32, tag="agg")
                    psZ = psump.tile([P, H], f32, tag="aggz")
                    emit_score(qb, 0, kvh2[0], A, E)
                    emit_score(qb, 1, kvh2[1], A, E)
                    emit_value(qb, 0, kvh2[0], E, psA, psZ)
                    if pend:
                        emit_epilogue(*pend.pop())
                    emit_value(qb, 1, kvh2[1], E, psA, psZ)
                    pend.append((qb, psA, psZ))
                emit_epilogue(*pend.pop())

    nc.compile()
    return nc


def _wrap_idx(lst):
    """int16 list -> [128, len/16] wrapped (pos i -> [i%16, i//16]) and
    replicated across the 8 groups of 16 partitions."""
    n = lst.shape[0]
    w = np.empty((P, n // 16), np.int16)
    blk = lst.reshape(n // 16, 16).T  # [16, n/16]
    for g in range(8):
        w[g * 16:(g + 1) * 16, :] = blk
    return w


def make_core_inputs(pairwise_g, coset_functions, nbhd_idx,
                     W_q, b_q, W_k, W_l, v, W_in, b_in, W_out, b_out):
    pairwise_g = np.asarray(pairwise_g)
    coset_functions = np.asarray(coset_functions)
    nbhd_idx = np.asarray(nbhd_idx).astype(np.int64)
    W_q = np.asarray(W_q, np.float32)
    b_q = np.asarray(b_q, np.float32)
    W_k = np.asarray(W_k, np.float32)
    W_l = np.asarray(W_l, np.float32)
    v = np.asarray(v, np.float32)
    W_in = np.asarray(W_in, np.float32)
    b_in = np.asarray(b_in, np.float32)
    W_out = np.asarray(W_out, np.float32)
    b_out = np.asarray(b_out, np.float32)

    wq16 = np.ascontiguousarray(
        W_q[CHMAP].T.reshape(2, P, C).astype(np.float16))
    wkv16 = np.ascontiguousarray(
        np.concatenate([W_k[CHMAP].T, W_in[CHMAP].T], axis=1)
        .reshape(2, P, CH2).astype(np.float16))
    wo16 = np.ascontiguousarray(
        W_out.T[CHMAP].reshape(2, P, C).astype(np.float16))
    wl_full = np.zeros((C, H * POS), np.float32)
    for h in range(H):
        for p_ in range(POS):
            wl_full[h * D:(h + 1) * D, p_ * H + h] = W_l[h * D:(h + 1) * D, p_]
    wl16 = np.ascontiguousarray(
        wl_full[CHMAP].reshape(2, P, H * POS).astype(np.float16))
    bqv32 = np.ascontiguousarray(
        (b_q + v.reshape(C))[CHMAP].reshape(2, P, 1).astype(np.float32))
    bqmat = np.ascontiguousarray(
        np.broadcast_to(b_q[CHMAP], (P, C)).astype(np.float32))
    bomat = np.ascontiguousarray(
        np.broadcast_to(W_out @ b_in + b_out, (P, C)).astype(np.float32))
    ident16 = np.eye(P, dtype=np.float16)

    in_maps = []
    for core in range(NCORES):
        b = core // 4
        qs = (core % 4) * NQ
        # roll keys so this core's queries are columns 0..NQ-1 of cosT
        # (the compiled program slices cosT[:, :, 0:NQ] for Q projections);
        # gather idx values are rotated to match the rolled KV table.
        cosT = np.ascontiguousarray(
            np.roll(coset_functions[b], -qs, axis=0)
            .T.reshape(2, P, N).astype(np.float16))
        idx = nbhd_idx[b, qs:qs + NQ]  # [NQ, M]
        idx_rot = (idx - qs) % N

        idxw = np.empty((P, QB, M * P // 16), np.int16)
        for qb in range(QB):
            blk = idx_rot[qb * P:(qb + 1) * P]  # [P(n), M]
            lst = blk.T.reshape(M * P)  # m-major: pos i = m*128 + n
            idxw[:, qb, :] = _wrap_idx(lst.astype(np.int16))

        # host-gathered pairwise_g neighbor rows, h-expanded, f16
        pg_rows = pairwise_g[b, qs + np.arange(NQ)[:, None], idx]  # [NQ,M,POS]
        pgx = np.broadcast_to(pg_rows[:, :, :, None].astype(np.float16),
                              (NQ, M, POS, H))
        pgx = np.ascontiguousarray(pgx).reshape(QB, P, M * POS * H)

        in_maps.append({
            "cosT": cosT,
            "wkv": wkv16, "wq": wq16, "wl": wl16, "wo": wo16,
            "bqv": bqv32, "bqmat": bqmat, "bomat": bomat,
            "ident": ident16, "idxw": idxw, "pgx": pgx,
        })
    return in_maps


def assemble_output(results):
    out = np.empty((B, N, C), np.float32)
    for core in range(NCORES):
        b = core // 4
        qs = (core % 4) * NQ
        o = results[core]["out"]  # [QB, P, C]
        out[b, qs:qs + NQ] = o.reshape(NQ, C)
    return out


def kernel(pairwise_g, coset_functions, mask, nbhd_idx,
           W_q, b_q, W_k, b_k, W_l, b_l, u, v,
           W_in, b_in, W_out, b_out, **_unused):
    from concourse.bass_utils import run_bass_kernel_spmd

    if "nc" not in _compiled:
        _compiled["nc"] = build_bass()
    nc = _compiled["nc"]

    in_maps = make_core_inputs(pairwise_g, coset_functions, nbhd_idx,
                               W_q, b_q, W_k, W_l, v, W_in, b_in,
                               W_out, b_out)
    res = run_bass_kernel_spmd(nc, in_maps, core_ids=list(range(NCORES)))
    return assemble_output(res.results)


# revision 75
# speedup vs baseline: 1.0868x; 1.0003x over previous
"""Trainium2 Bass kernel for nn_EquivairantMultiheadAttention (sparse attention).

Contract: kernel(**inputs) takes the FULL unsharded numpy inputs (as produced by
setup_inputs()) and returns the FULL (B, N, COUT) float32 output.

Sharding: 8 cores = data-parallel over batch (2) x sequence-parallel over the
query dim n (4 slices of 512).

Architecture (v1, f16):
 - All PE inputs and gathered tables in float16; channel order permuted to
   d-major (ch' = d*8 + h) so per-head broadcasts have contiguous last dims
   (enables the DVE 2x 16-bit mode on every big elementwise op).
 - K and V rows stored interleaved in one DRAM table (1KB rows); one
   dma_gather per 1024 neighbors fetches both (half the descriptors).
 - pairwise_g neighbor rows are gathered on HOST (pure data selection) and
   streamed as a dense (512, 64, 6, 8) f16 tensor (h-expanded for 2x mode).
 - Scores: t1 = kg*Q elementwise + in-place halving tree over d (all f16 2x);
   t2 = pgx*G + tree; t3/b_k/b_l terms are constant over the softmax axis m
   and drop out. Softmax exp on the Act engine; normalization deferred.
 - Aggregation: per-m matmuls with identity lhsT accumulate E-scaled V rows
   in PSUM (replaces a DVE reduction tree); 1/Z applied on PSUM evac;
 - b_in folded into b_out on host: out = W_out@agg + (W_out@b_in + b_out).
"""

import math
import sys

import numpy as np

sys.path.insert(0, "/opt/trn_rl_repo")

B, N, M = 2, 2048, 64
C = 256  # CIN == COUT
H, D, POS = 8, 32, 6
NQ = 512  # queries per core
QB = 4  # query blocks of 128 per core
P = 128
NCORES = 8
INV_SQRT_D = 1.0 / math.sqrt(D)
NIC = 1024  # max idxs per dma_gather call (HW: larger crashes the exec unit)
CH2 = 2 * C  # combined K|V row width

_compiled = {}

# channel permutation: ch' = d*8 + h  <->  ch = h*32 + d
CHMAP = np.array([(cp % 8) * D + (cp // 8) for cp in range(C)], dtype=np.int64)


def build_bass():
    import concourse.bacc as bacc
    import concourse.mybir as mybir
    import concourse.tile as tile

    dt = mybir.dt
    nc = bacc.Bacc("TRN2", target_bir_lowering=False, debug=False,
                   enable_asserts=False, num_devices=NCORES)

    f16 = dt.float16
    f32 = dt.float32
    i16 = dt.int16

    # ---- DRAM inputs (per core) ----
    # cosT is rolled per-core so this core's NQ query columns are 0..NQ-1
    # (gather idx values are rotated to match on host).
    d_cosT = nc.dram_tensor("cosT", [2, P, N], f16, kind="ExternalInput")
    d_wkv = nc.dram_tensor("wkv", [2, P, CH2], f16, kind="ExternalInput")
    d_wq = nc.dram_tensor("wq", [2, P, C], f16, kind="ExternalInput")
    d_wl = nc.dram_tensor("wl", [2, P, H * POS], f16, kind="ExternalInput")
    d_wo = nc.dram_tensor("wo", [2, P, C], f16, kind="ExternalInput")
    d_bqv = nc.dram_tensor("bqv", [2, P, 1], f32, kind="ExternalInput")
    d_bqmat = nc.dram_tensor("bqmat", [P, C], f32, kind="ExternalInput")
    d_bomat = nc.dram_tensor("bomat", [P, C], f32, kind="ExternalInput")
    d_ident = nc.dram_tensor("ident", [P, P], f16, kind="ExternalInput")
    d_idxw = nc.dram_tensor("idxw", [P, QB, M * P // 16], i16,
                            kind="ExternalInput")
    d_pgx = nc.dram_tensor("pgx", [QB, P, M * POS * H], f16,
                           kind="ExternalInput")
    d_out = nc.dram_tensor("out", [QB, P, C], f32, kind="ExternalOutput")

    add = mybir.AluOpType.add
    mult = mybir.AluOpType.mult

    with tile.TileContext(nc) as tc:
        with (
            tc.tile_pool(name="const", bufs=1) as constp,
            tc.tile_pool(name="dram", bufs=1, space="DRAM") as dramp,
            tc.tile_pool(name="psum", bufs=2, space="PSUM") as psump,
            tc.tile_pool(name="evac", bufs=2) as evacp,
        ):
            # critical-path loads first: cosT gates the KV table -> gathers
            cosT = constp.tile([P, 2, N], f16)
            wkv = constp.tile([P, 2, CH2], f16)
            ident = constp.tile([P, P], f16)
            for cc in range(2):
                nc.sync.dma_start(cosT[:, cc, :], d_cosT[cc])
                nc.sync.dma_start(wkv[:, cc, :], d_wkv[cc])
            nc.sync.dma_start(ident[:], d_ident.ap())
            wq = constp.tile([P, 2, C], f16)
            wl = constp.tile([P, 2, H * POS], f16)
            wo = constp.tile([P, 2, C], f16)
            for cc in range(2):
                nc.sync.dma_start(wq[:, cc, :], d_wq[cc])
                nc.sync.dma_start(wl[:, cc, :], d_wl[cc])
                nc.sync.dma_start(wo[:, cc, :], d_wo[cc])
            bqv = constp.tile([P, 2, 1], f32)
            nc.sync.dma_start(bqv[:], d_bqv.ap().rearrange("c p one -> p c one"))
            bqmat = constp.tile([P, C], f32)
            bomat = constp.tile([P, C], f32)
            nc.sync.dma_start(bqmat[:], d_bqmat.ap())
            nc.sync.dma_start(bomat[:], d_bomat.ap())
            idxw = constp.tile([P, QB, M * P // 16], i16)
            nc.sync.dma_start(idxw[:], d_idxw.ap())
            nbias = constp.tile([P, 1], f32)
            nc.vector.memset(nbias[:], -4.0)

            # combined K|V rows (f16, permuted channels) in DRAM scratch
            kvdr = dramp.tile([N, CH2], f16)

            q_rows = constp.tile([P, QB, C], f16)
            g_rows = constp.tile([P, QB, H * POS], f16)

            with tc.tile_pool(name="prep", bufs=1) as prepp:
                # tiny matmuls advance the PE pstate ramp almost for free so
                # the KV-table matmuls run at speed
                psW = psump.tile([P, 2], f32, tag="pst", bufs=1)
                for i in range(12):
                    nc.tensor.matmul(psW[:], lhsT=ident[:], rhs=ident[:, 0:2],
                                     start=(i == 0), stop=(i == 11))
                JB = 8  # jt rows per batched kvdr write
                for jg in range(N // P // JB):
                    kv_sb = evacp.tile([P, JB, CH2], f16, tag="kvevac")
                    for j in range(JB):
                        jt = jg * JB + j
                        ps = psump.tile([P, CH2], f32, tag="ps", bufs=3)
                        for cc in range(2):
                            nc.tensor.matmul(
                                ps[:],
                                lhsT=cosT[:, cc, jt * P:(jt + 1) * P],
                                rhs=wkv[:, cc, :],
                                start=(cc == 0), stop=(cc == 1))
                        nc.vector.tensor_copy(kv_sb[:, j, :], ps[:])
                    nc.sync.dma_start(
                        kvdr[jg * JB * P:(jg + 1) * JB * P, :]
                        .rearrange("(j p) w -> p j w", j=JB),
                        kv_sb[:])

                for nt in range(QB):
                    ps = psump.tile([P, C], f32, tag="agg", bufs=2)
                    for cc in range(2):
                        nc.tensor.matmul(ps[:],
                                         lhsT=cosT[:, cc, nt * P:(nt + 1) * P],
                                         rhs=wq[:, cc, :],
                                         start=(cc == 0), stop=(cc == 1))
                    nc.vector.tensor_tensor(out=q_rows[:, nt, :], in0=ps[:],
                                            in1=bqmat[:], op=add)

                qvT = prepp.tile([P, 2, NQ], f16)
                for cc2 in range(2):
                    ps = psump.tile([P, NQ], f32, tag="agg", bufs=2)
                    for cc in range(2):
                        nc.tensor.matmul(ps[:],
                                         lhsT=wq[:, cc, cc2 * P:(cc2 + 1) * P],
                                         rhs=cosT[:, cc, 0:NQ],
                                         start=(cc == 0), stop=(cc == 1))
                    nc.vector.tensor_tensor(
                        out=qvT[:, cc2, :], in0=ps[:],
                        in1=bqv[:, cc2, :].broadcast_to([P, NQ]), op=add)
                for nt in range(QB):
                    ps = psump.tile([P, H * POS], f32, tag="agg", bufs=2)
                    for cc in range(2):
                        nc.tensor.matmul(ps[:],
                                         lhsT=qvT[:, cc, nt * P:(nt + 1) * P],
                                         rhs=wl[:, cc, :],
                                         start=(cc == 0), stop=(cc == 1))
                    nc.scalar.copy(g_rows[:, nt, :], ps[:])

            # ---- main loop: per query block ----
            with (
                tc.tile_pool(name="kvp", bufs=4) as kvp,
                tc.tile_pool(name="pgp", bufs=4) as pgp,
                tc.tile_pool(name="ap", bufs=2) as app,
                tc.tile_pool(name="small", bufs=2) as smallp,
            ):
                MH = M // 2

                def emit_score(qb, mh, kvh, A, E):
                    # chunked at gather-call granularity (8 m per call) so
                    # scoring starts as soon as the first call lands
                    ms = slice(mh * MH, (mh + 1) * MH)
                    CHK = 16
                    for ck in range(MH // CHK):
                        cs0 = mh * MH + ck * CHK
                        kg = (kvh[:, ck * CHK:(ck + 1) * CHK, 0:C]
                              .rearrange("p m (d h) -> p m d h", h=H))
                        nc.vector.tensor_tensor(
                            out=kg, in0=kg,
                            in1=q_rows[:, qb, :][:, None, :]
                                .broadcast_to([P, CHK, C])
                                .rearrange("p m (d h) -> p m d h", h=H),
                            op=mult)
                        dd = D // 2
                        while dd >= 1:
                            nc.vector.tensor_tensor(out=kg[:, :, 0:dd, :],
                                                    in0=kg[:, :, 0:dd, :],
                                                    in1=kg[:, :, dd:2 * dd, :],
                                                    op=add)
                            dd //= 2
                        nc.vector.tensor_tensor(
                            out=A[:, cs0:cs0 + CHK, :],
                            in0=A[:, cs0:cs0 + CHK, :],
                            in1=kg[:, :, 0, :], op=add)
                        nc.scalar.activation(
                            out=E[:, cs0:cs0 + CHK, :],
                            in_=A[:, cs0:cs0 + CHK, :],
                            func=mybir.ActivationFunctionType.Exp,
                            scale=INV_SQRT_D, bias=nbias[:])

                def emit_value(qb, mh, kvh, E, psA, psZ):
                    # Z via tiny identity matmuls: real work that also
                    # advances the PE pstate ramp (they only need E, not vg)
                    for mm in range(MH):
                        m0 = mh * MH + mm
                        nc.tensor.matmul(
                            psZ[:], lhsT=ident[:], rhs=E[:, m0, :],
                            start=(mh == 0 and mm == 0),
                            stop=(mh == 1 and mm == MH - 1),
                            skip_group_check=True)
                    # chunk the E-scaling so the first aggregation matmuls
                    # start ~3us earlier and PE stays continuously busy
                    CHK = 8
                    for ck in range(MH // CHK):
                        cs0 = mh * MH + ck * CHK
                        vgc = (kvh[:, ck * CHK:(ck + 1) * CHK, C:CH2]
                               .rearrange("p m (d h) -> p m d h", h=H))
                        nc.vector.tensor_tensor(
                            out=vgc, in0=vgc,
                            in1=E[:, cs0:cs0 + CHK, None, :]
                                .broadcast_to([P, CHK, D, H]),
                            op=mult)
                        for mm in range(ck * CHK, (ck + 1) * CHK):
                            nc.tensor.matmul(
                                psA[:], lhsT=ident[:],
                                rhs=kvh[:, mm, C:CH2],
                                start=(mh == 0 and mm == 0),
                                stop=(mh == 1 and mm == MH - 1),
                                skip_group_check=True)

                def emit_epilogue(qb, psA, psZ):
                    # fold psum halves, normalize (1/Z); transpose + out proj
                    rz = smallp.tile([P, H], f32, tag="rz", name=f"rz{qb}")
                    nc.vector.reciprocal(rz[:], psZ[:])
                    aggN = smallp.tile([P, C], f16, tag="aggN",
                                       name=f"aggN{qb}")
                    nc.vector.tensor_tensor(
                        out=aggN[:].rearrange("p (d h) -> p d h", h=H),
                        in0=psA[:].rearrange("p (d h) -> p d h", h=H),
                        in1=rz[:][:, None, :].broadcast_to([P, D, H]),
                        op=mult)
                    aggT = smallp.tile([P, 2, P], f16, tag="aggT",
                                       name=f"aggT{qb}")
                    for cc in range(2):
                        pst = psump.tile([P, P], f16, tag="pst", bufs=1,
                                         name=f"pst{qb}_{cc}")
                        nc.tensor.transpose(pst[:],
                                            aggN[:, cc * P:(cc + 1) * P],
                                            ident[:])
                        nc.scalar.copy(aggT[:, cc, :], pst[:])
                    psO = psump.tile([P, C], f32, tag="pst", bufs=1,
                                     name=f"psO{qb}")
                    for cc in range(2):
                        nc.tensor.matmul(psO[:], lhsT=aggT[:, cc, :],
                                         rhs=wo[:, cc, :],
                                         start=(cc == 0), stop=(cc == 1))
                    out_sb = smallp.tile([P, C], f32, tag="outsb",
                                         name=f"outsb{qb}")
                    nc.vector.tensor_tensor(out=out_sb[:], in0=psO[:],
                                            in1=bomat[:], op=add)
                    nc.sync.dma_start(d_out[qb], out_sb[:])

                pgx_tiles = {}
                pend = []  # deferred epilogue args (software pipelining)
                for qb in range(QB):
                    pgx_tiles[qb] = pgp.tile([P, M, POS, H], f16, tag="pgx",
                                             name=f"pgx{qb}")
                    nc.sync.dma_start(pgx_tiles[qb][:], d_pgx[qb])
                for qb in range(QB):
                    kvh2 = []
                    for mh in range(2):
                        kvh = kvp.tile([P, MH, CH2], f16, tag="kv",
                                       name=f"kv{qb}_{mh}")
                        kvh2.append(kvh)
                        for kk in range(4):
                            k = mh * 4 + kk
                            nc.gpsimd.dma_gather(
                                out_ap=kvh[:, kk * (NIC // P):
                                           (kk + 1) * (NIC // P), :],
                                in_ap=kvdr[:],
                                idxs_ap=idxw[:, qb, k * (NIC // 16):
                                             (k + 1) * (NIC // 16)],
                                num_idxs=NIC, num_idxs_reg=NIC,
                                elem_size=CH2)

                    # t2 -> A (rel-pos term); pgx layout [P, M, POS, H]
                    pgx = pgx_tiles[qb]
                    A = app.tile([P, M, H], f16, tag="A")
                    gb = (g_rows[:, qb, :]
                          .rearrange("p (pp h) -> p pp h", h=H)
                          [:, None, :, :].broadcast_to([P, M, POS, H]))
                    nc.vector.tensor_tensor(out=pgx[:], in0=pgx[:], in1=gb,
                                            op=mult)
                    nc.vector.tensor_tensor(out=pgx[:, :, 0:3, :],
                                            in0=pgx[:, :, 0:3, :],
                                            in1=pgx[:, :, 3:6, :], op=add)
                    nc.vector.tensor_tensor(out=pgx[:, :, 0:1, :],
                                            in0=pgx[:, :, 0:1, :],
                                            in1=pgx[:, :, 1:2, :], op=add)
                    nc.vector.tensor_tensor(out=A[:], in0=pgx[:, :, 0, :],
                                            in1=pgx[:, :, 2, :], op=add)

                    # per m-half: t1 scores, exp (fixed -4 bias keeps f16 E*V
                    # in range; softmax shift-invariant, 1/Z deferred), V
                    # scaling, and PE identity-accumulation — pipelined so PE
                    # aggregation of half 0 overlaps DVE scoring of half 1.
                    # The previous qblock's epilogue is emitted between the
                    # halves so its PSUM reads never stall this one's DVE.
                    E = smallp.tile([P, M, H], f16, tag="E")
                    psA = psump.tile([P, C], f32, tag="agg")
                    psZ = psump.tile([P, H], f32, tag="aggz")
                    emit_score(qb, 0, kvh2[0], A, E)
                    emit_score(qb, 1, kvh2[1], A, E)
                    emit_value(qb, 0, kvh2[0], E, psA, psZ)
                    if pend:
                        emit_epilogue(*pend.pop())
                    emit_value(qb, 1, kvh2[1], E, psA, psZ)
                    pend.append((qb, psA, psZ))
                emit_epilogue(*pend.pop())

    nc.compile()
    return nc


def _wrap_idx(lst):
    """int16 list -> [128, len/16] wrapped (pos i -> [i%16, i//16]) and
    replicated across the 8 groups of 16 partitions."""
    n = lst.shape[0]
    w = np.empty((P, n // 16), np.int16)
    blk = lst.reshape(n // 16, 16).T  # [16, n/16]
    for g in range(8):
        w[g * 16:(g + 1) * 16, :] = blk
    return w


def make_core_inputs(pairwise_g, coset_functions, nbhd_idx,
                     W_q, b_q, W_k, W_l, v, W_in, b_in, W_out, b_out):
    pairwise_g = np.asarray(pairwise_g)
    coset_functions = np.asarray(coset_functions)
    nbhd_idx = np.asarray(nbhd_idx).astype(np.int64)
    W_q = np.asarray(W_q, np.float32)
    b_q = np.asarray(b_q, np.float32)
    W_k = np.asarray(W_k, np.float32)
    W_l = np.asarray(W_l, np.float32)
    v = np.asarray(v, np.float32)
    W_in = np.asarray(W_in, np.float32)
    b_in = np.asarray(b_in, np.float32)
    W_out = np.asarray(W_out, np.float32)
    b_out = np.asarray(b_out, np.float32)

    wq16 = np.ascontiguousarray(
        W_q[CHMAP].T.reshape(2, P, C).astype(np.float16))
    wkv16 = np.ascontiguousarray(
        np.concatenate([W_k[CHMAP].T, W_in[CHMAP].T], axis=1)
        .reshape(2, P, CH2).astype(np.float16))
    wo16 = np.ascontiguousarray(
        W_out.T[CHMAP].reshape(2, P, C).astype(np.float16))
    wl_full = np.zeros((C, H * POS), np.float32)
    for h in range(H):
        for p_ in range(POS):
            wl_full[h * D:(h + 1) * D, p_ * H + h] = W_l[h * D:(h + 1) * D, p_]
    wl16 = np.ascontiguousarray(
        wl_full[CHMAP].reshape(2, P, H * POS).astype(np.float16))
    bqv32 = np.ascontiguousarray(
        (b_q + v.reshape(C))[CHMAP].reshape(2, P, 1).astype(np.float32))
    bqmat = np.ascontiguousarray(
        np.broadcast_to(b_q[CHMAP], (P, C)).astype(np.float32))
    bomat = np.ascontiguousarray(
        np.broadcast_to(W_out @ b_in + b_out, (P, C)).astype(np.float32))
    ident16 = np.eye(P, dtype=np.float16)

    in_maps = []
    for core in range(NCORES):
        b = core // 4
        qs = (core % 4) * NQ
        # roll keys so this core's queries are columns 0..NQ-1 of cosT
        # (the compiled program slices cosT[:, :, 0:NQ] for Q projections);
        # gather idx values are rotated to match the rolled KV table.
        cosT = np.ascontiguousarray(
            np.roll(coset_functions[b], -qs, axis=0)
            .T.reshape(2, P, N).astype(np.float16))
        idx = nbhd_idx[b, qs:qs + NQ]  # [NQ, M]
        idx_rot = (idx - qs) % N

        idxw = np.empty((P, QB, M * P // 16), np.int16)
        for qb in range(QB):
            blk = idx_rot[qb * P:(qb + 1) * P]  # [P(n), M]
            lst = blk.T.reshape(M * P)  # m-major: pos i = m*128 + n
            idxw[:, qb, :] = _wrap_idx(lst.astype(np.int16))

        # host-gathered pairwise_g neighbor rows, h-expanded, f16
        pg_rows = pairwise_g[b, qs + np.arange(NQ)[:, None], idx]  # [NQ,M,POS]
        pgx = np.broadcast_to(pg_rows[:, :, :, None].astype(np.float16),
                              (NQ, M, POS, H))
        pgx = np.ascontiguousarray(pgx).reshape(QB, P, M * POS * H)

        in_maps.append({
            "cosT": cosT,
            "wkv": wkv16, "wq": wq16, "wl": wl16, "wo": wo16,
            "bqv": bqv32, "bqmat": bqmat, "bomat": bomat,
            "ident": ident16, "idxw": idxw, "pgx": pgx,
        })
    return in_maps


def assemble_output(results):
    out = np.empty((B, N, C), np.float32)
    for core in range(NCORES):
        b = core // 4
        qs = (core % 4) * NQ
        o = results[core]["out"]  # [QB, P, C]
        out[b, qs:qs + NQ] = o.reshape(NQ, C)
    return out


def kernel(pairwise_g, coset_functions, mask, nbhd_idx,
           W_q, b_q, W_k, b_k, W_l, b_l, u, v,
           W_in, b_in, W_out, b_out, **_unused):
    from concourse.bass_utils import run_bass_kernel_spmd

    if "nc" not in _compiled:
        _compiled["nc"] = build_bass()
    nc = _compiled["nc"]

    in_maps = make_core_inputs(pairwise_g, coset_functions, nbhd_idx,
                               W_q, b_q, W_k, W_l, v, W_in, b_in,
                               W_out, b_out)
    res = run_bass_kernel_spmd(nc, in_maps, core_ids=list(range(NCORES)))
    return assemble_output(res.results)


# revision 76
# speedup vs baseline: 1.0910x; 1.0039x over previous
"""Trainium2 Bass kernel for nn_EquivairantMultiheadAttention (sparse attention).

Contract: kernel(**inputs) takes the FULL unsharded numpy inputs (as produced by
setup_inputs()) and returns the FULL (B, N, COUT) float32 output.

Sharding: 8 cores = data-parallel over batch (2) x sequence-parallel over the
query dim n (4 slices of 512).

Architecture (v1, f16):
 - All PE inputs and gathered tables in float16; channel order permuted to
   d-major (ch' = d*8 + h) so per-head broadcasts have contiguous last dims
   (enables the DVE 2x 16-bit mode on every big elementwise op).
 - K and V rows stored interleaved in one DRAM table (1KB rows); one
   dma_gather per 1024 neighbors fetches both (half the descriptors).
 - pairwise_g neighbor rows are gathered on HOST (pure data selection) and
   streamed as a dense (512, 64, 6, 8) f16 tensor (h-expanded for 2x mode).
 - Scores: t1 = kg*Q elementwise + in-place halving tree over d (all f16 2x);
   t2 = pgx*G + tree; t3/b_k/b_l terms are constant over the softmax axis m
   and drop out. Softmax exp on the Act engine; normalization deferred.
 - Aggregation: per-m matmuls with identity lhsT accumulate E-scaled V rows
   in PSUM (replaces a DVE reduction tree); 1/Z applied on PSUM evac;
 - b_in folded into b_out on host: out = W_out@agg + (W_out@b_in + b_out).
"""

import math
import sys

import numpy as np

sys.path.insert(0, "/opt/trn_rl_repo")

B, N, M = 2, 2048, 64
C = 256  # CIN == COUT
H, D, POS = 8, 32, 6
NQ = 512  # queries per core
QB = 4  # query blocks of 128 per core
P = 128
NCORES = 8
INV_SQRT_D = 1.0 / math.sqrt(D)
NIC = 1024  # max idxs per dma_gather call (HW: larger crashes the exec unit)
CH2 = 2 * C  # combined K|V row width

_compiled = {}

# channel permutation: ch' = d*8 + h  <->  ch = h*32 + d
CHMAP = np.array([(cp % 8) * D + (cp // 8) for cp in range(C)], dtype=np.int64)


def build_bass():
    import concourse.bacc as bacc
    import concourse.mybir as mybir
    import concourse.tile as tile

    dt = mybir.dt
    nc = bacc.Bacc("TRN2", target_bir_lowering=False, debug=False,
                   enable_asserts=False, num_devices=NCORES)

    f16 = dt.float16
    f32 = dt.float32
    i16 = dt.int16

    # ---- DRAM inputs (per core) ----
    # cosT is rolled per-core so this core's NQ query columns are 0..NQ-1
    # (gather idx values are rotated to match on host).
    d_cosT = nc.dram_tensor("cosT", [2, P, N], f16, kind="ExternalInput")
    d_wkv = nc.dram_tensor("wkv", [2, P, CH2], f16, kind="ExternalInput")
    d_wq = nc.dram_tensor("wq", [2, P, C], f16, kind="ExternalInput")
    d_wl = nc.dram_tensor("wl", [2, P, H * POS], f16, kind="ExternalInput")
    d_wo = nc.dram_tensor("wo", [2, P, C], f16, kind="ExternalInput")
    d_bqv = nc.dram_tensor("bqv", [2, P, 1], f32, kind="ExternalInput")
    d_bqmat = nc.dram_tensor("bqmat", [P, C], f32, kind="ExternalInput")
    d_bomat = nc.dram_tensor("bomat", [P, C], f32, kind="ExternalInput")
    d_ident = nc.dram_tensor("ident", [P, P], f16, kind="ExternalInput")
    d_idxw = nc.dram_tensor("idxw", [P, QB, M * P // 16], i16,
                            kind="ExternalInput")
    d_pgx = nc.dram_tensor("pgx", [QB, P, M * POS * H], f16,
                           kind="ExternalInput")
    d_out = nc.dram_tensor("out", [QB, P, C], f32, kind="ExternalOutput")

    add = mybir.AluOpType.add
    mult = mybir.AluOpType.mult

    with tile.TileContext(nc) as tc:
        with (
            tc.tile_pool(name="const", bufs=1) as constp,
            tc.tile_pool(name="dram", bufs=1, space="DRAM") as dramp,
            tc.tile_pool(name="psum", bufs=2, space="PSUM") as psump,
            tc.tile_pool(name="evac", bufs=2) as evacp,
        ):
            # critical-path loads first: cosT gates the KV table -> gathers
            cosT = constp.tile([P, 2, N], f16)
            wkv = constp.tile([P, 2, CH2], f16)
            ident = constp.tile([P, P], f16)
            for cc in range(2):
                nc.sync.dma_start(cosT[:, cc, :], d_cosT[cc])
                nc.sync.dma_start(wkv[:, cc, :], d_wkv[cc])
            nc.sync.dma_start(ident[:], d_ident.ap())
            wq = constp.tile([P, 2, C], f16)
            wl = constp.tile([P, 2, H * POS], f16)
            wo = constp.tile([P, 2, C], f16)
            for cc in range(2):
                nc.sync.dma_start(wq[:, cc, :], d_wq[cc])
                nc.sync.dma_start(wl[:, cc, :], d_wl[cc])
                nc.sync.dma_start(wo[:, cc, :], d_wo[cc])
            bqv = constp.tile([P, 2, 1], f32)
            nc.sync.dma_start(bqv[:], d_bqv.ap().rearrange("c p one -> p c one"))
            bqmat = constp.tile([P, C], f32)
            bomat = constp.tile([P, C], f32)
            nc.sync.dma_start(bqmat[:], d_bqmat.ap())
            nc.sync.dma_start(bomat[:], d_bomat.ap())
            idxw = constp.tile([P, QB, M * P // 16], i16)
            nc.sync.dma_start(idxw[:], d_idxw.ap())
            nbias = constp.tile([P, 1], f32)
            nc.vector.memset(nbias[:], -4.0)

            # combined K|V rows (f16, permuted channels) in DRAM scratch
            kvdr = dramp.tile([N, CH2], f16)

            q_rows = constp.tile([P, QB, C], f16)
            g_rows = constp.tile([P, QB, H * POS], f16)

            with tc.tile_pool(name="prep", bufs=1) as prepp:
                # tiny matmuls advance the PE pstate ramp almost for free so
                # the KV-table matmuls run at speed
                psW = psump.tile([P, 2], f32, tag="pst", bufs=1)
                for i in range(12):
                    nc.tensor.matmul(psW[:], lhsT=ident[:], rhs=ident[:, 0:2],
                                     start=(i == 0), stop=(i == 11))
                JB = 8  # jt rows per batched kvdr write
                for jg in range(N // P // JB):
                    kv_sb = evacp.tile([P, JB, CH2], f16, tag="kvevac")
                    for j in range(JB):
                        jt = jg * JB + j
                        ps = psump.tile([P, CH2], f32, tag="ps", bufs=3)
                        for cc in range(2):
                            nc.tensor.matmul(
                                ps[:],
                                lhsT=cosT[:, cc, jt * P:(jt + 1) * P],
                                rhs=wkv[:, cc, :],
                                start=(cc == 0), stop=(cc == 1))
                        nc.scalar.copy(kv_sb[:, j, :], ps[:])
                    nc.sync.dma_start(
                        kvdr[jg * JB * P:(jg + 1) * JB * P, :]
                        .rearrange("(j p) w -> p j w", j=JB),
                        kv_sb[:])

                for nt in range(QB):
                    ps = psump.tile([P, C], f32, tag="agg", bufs=2)
                    for cc in range(2):
                        nc.tensor.matmul(ps[:],
                                         lhsT=cosT[:, cc, nt * P:(nt + 1) * P],
                                         rhs=wq[:, cc, :],
                                         start=(cc == 0), stop=(cc == 1))
                    nc.vector.tensor_tensor(out=q_rows[:, nt, :], in0=ps[:],
                                            in1=bqmat[:], op=add)

                qvT = prepp.tile([P, 2, NQ], f16)
                for cc2 in range(2):
                    ps = psump.tile([P, NQ], f32, tag="agg", bufs=2)
                    for cc in range(2):
                        nc.tensor.matmul(ps[:],
                                         lhsT=wq[:, cc, cc2 * P:(cc2 + 1) * P],
                                         rhs=cosT[:, cc, 0:NQ],
                                         start=(cc == 0), stop=(cc == 1))
                    nc.vector.tensor_tensor(
                        out=qvT[:, cc2, :], in0=ps[:],
                        in1=bqv[:, cc2, :].broadcast_to([P, NQ]), op=add)
                for nt in range(QB):
                    ps = psump.tile([P, H * POS], f32, tag="agg", bufs=2)
                    for cc in range(2):
                        nc.tensor.matmul(ps[:],
                                         lhsT=qvT[:, cc, nt * P:(nt + 1) * P],
                                         rhs=wl[:, cc, :],
                                         start=(cc == 0), stop=(cc == 1))
                    nc.scalar.copy(g_rows[:, nt, :], ps[:])

            # ---- main loop: per query block ----
            with (
                tc.tile_pool(name="kvp", bufs=4) as kvp,
                tc.tile_pool(name="pgp", bufs=4) as pgp,
                tc.tile_pool(name="ap", bufs=2) as app,
                tc.tile_pool(name="small", bufs=2) as smallp,
            ):
                MH = M // 2

                def emit_score(qb, mh, kvh, A, E):
                    # chunked at gather-call granularity (8 m per call) so
                    # scoring starts as soon as the first call lands
                    ms = slice(mh * MH, (mh + 1) * MH)
                    CHK = 16
                    for ck in range(MH // CHK):
                        cs0 = mh * MH + ck * CHK
                        kg = (kvh[:, ck * CHK:(ck + 1) * CHK, 0:C]
                              .rearrange("p m (d h) -> p m d h", h=H))
                        nc.vector.tensor_tensor(
                            out=kg, in0=kg,
                            in1=q_rows[:, qb, :][:, None, :]
                                .broadcast_to([P, CHK, C])
                                .rearrange("p m (d h) -> p m d h", h=H),
                            op=mult)
                        dd = D // 2
                        while dd >= 1:
                            nc.vector.tensor_tensor(out=kg[:, :, 0:dd, :],
                                                    in0=kg[:, :, 0:dd, :],
                                                    in1=kg[:, :, dd:2 * dd, :],
                                                    op=add)
                            dd //= 2
                        nc.vector.tensor_tensor(
                            out=A[:, cs0:cs0 + CHK, :],
                            in0=A[:, cs0:cs0 + CHK, :],
                            in1=kg[:, :, 0, :], op=add)
                        nc.scalar.activation(
                            out=E[:, cs0:cs0 + CHK, :],
                            in_=A[:, cs0:cs0 + CHK, :],
                            func=mybir.ActivationFunctionType.Exp,
                            scale=INV_SQRT_D, bias=nbias[:])

                def emit_value(qb, mh, kvh, E, psA, psZ):
                    # Z via tiny identity matmuls: real work that also
                    # advances the PE pstate ramp (they only need E, not vg)
                    for mm in range(MH):
                        m0 = mh * MH + mm
                        nc.tensor.matmul(
                            psZ[:], lhsT=ident[:], rhs=E[:, m0, :],
                            start=(mh == 0 and mm == 0),
                            stop=(mh == 1 and mm == MH - 1),
                            skip_group_check=True)
                    # chunk the E-scaling so the first aggregation matmuls
                    # start ~3us earlier and PE stays continuously busy
                    CHK = 8
                    for ck in range(MH // CHK):
                        cs0 = mh * MH + ck * CHK
                        vgc = (kvh[:, ck * CHK:(ck + 1) * CHK, C:CH2]
                               .rearrange("p m (d h) -> p m d h", h=H))
                        nc.vector.tensor_tensor(
                            out=vgc, in0=vgc,
                            in1=E[:, cs0:cs0 + CHK, None, :]
                                .broadcast_to([P, CHK, D, H]),
                            op=mult)
                        for mm in range(ck * CHK, (ck + 1) * CHK):
                            nc.tensor.matmul(
                                psA[:], lhsT=ident[:],
                                rhs=kvh[:, mm, C:CH2],
                                start=(mh == 0 and mm == 0),
                                stop=(mh == 1 and mm == MH - 1),
                                skip_group_check=True)

                def emit_epilogue(qb, psA, psZ):
                    # fold psum halves, normalize (1/Z); transpose + out proj
                    rz = smallp.tile([P, H], f32, tag="rz", name=f"rz{qb}")
                    nc.vector.reciprocal(rz[:], psZ[:])
                    aggN = smallp.tile([P, C], f16, tag="aggN",
                                       name=f"aggN{qb}")
                    nc.vector.tensor_tensor(
                        out=aggN[:].rearrange("p (d h) -> p d h", h=H),
                        in0=psA[:].rearrange("p (d h) -> p d h", h=H),
                        in1=rz[:][:, None, :].broadcast_to([P, D, H]),
                        op=mult)
                    aggT = smallp.tile([P, 2, P], f16, tag="aggT",
                                       name=f"aggT{qb}")
                    for cc in range(2):
                        pst = psump.tile([P, P], f16, tag="pst", bufs=1,
                                         name=f"pst{qb}_{cc}")
                        nc.tensor.transpose(pst[:],
                                            aggN[:, cc * P:(cc + 1) * P],
                                            ident[:])
                        nc.scalar.copy(aggT[:, cc, :], pst[:])
                    psO = psump.tile([P, C], f32, tag="pst", bufs=1,
                                     name=f"psO{qb}")
                    for cc in range(2):
                        nc.tensor.matmul(psO[:], lhsT=aggT[:, cc, :],
                                         rhs=wo[:, cc, :],
                                         start=(cc == 0), stop=(cc == 1))
                    out_sb = smallp.tile([P, C], f32, tag="outsb",
                                         name=f"outsb{qb}")
                    nc.vector.tensor_tensor(out=out_sb[:], in0=psO[:],
                                            in1=bomat[:], op=add)
                    nc.sync.dma_start(d_out[qb], out_sb[:])

                pgx_tiles = {}
                pend = []  # deferred epilogue args (software pipelining)
                for qb in range(QB):
                    pgx_tiles[qb] = pgp.tile([P, M, POS, H], f16, tag="pgx",
                                             name=f"pgx{qb}")
                    nc.sync.dma_start(pgx_tiles[qb][:], d_pgx[qb])
                for qb in range(QB):
                    kvh2 = []
                    for mh in range(2):
                        kvh = kvp.tile([P, MH, CH2], f16, tag="kv",
                                       name=f"kv{qb}_{mh}")
                        kvh2.append(kvh)
                        for kk in range(4):
                            k = mh * 4 + kk
                            nc.gpsimd.dma_gather(
                                out_ap=kvh[:, kk * (NIC // P):
                                           (kk + 1) * (NIC // P), :],
                                in_ap=kvdr[:],
                                idxs_ap=idxw[:, qb, k * (NIC // 16):
                                             (k + 1) * (NIC // 16)],
                                num_idxs=NIC, num_idxs_reg=NIC,
                                elem_size=CH2)

                    # t2 -> A (rel-pos term); pgx layout [P, M, POS, H]
                    pgx = pgx_tiles[qb]
                    A = app.tile([P, M, H], f16, tag="A")
                    gb = (g_rows[:, qb, :]
                          .rearrange("p (pp h) -> p pp h", h=H)
                          [:, None, :, :].broadcast_to([P, M, POS, H]))
                    nc.vector.tensor_tensor(out=pgx[:], in0=pgx[:], in1=gb,
                                            op=mult)
                    nc.vector.tensor_tensor(out=pgx[:, :, 0:3, :],
                                            in0=pgx[:, :, 0:3, :],
                                            in1=pgx[:, :, 3:6, :], op=add)
                    nc.vector.tensor_tensor(out=pgx[:, :, 0:1, :],
                                            in0=pgx[:, :, 0:1, :],
                                            in1=pgx[:, :, 1:2, :], op=add)
                    nc.vector.tensor_tensor(out=A[:], in0=pgx[:, :, 0, :],
                                            in1=pgx[:, :, 2, :], op=add)

                    # per m-half: t1 scores, exp (fixed -4 bias keeps f16 E*V
                    # in range; softmax shift-invariant, 1/Z deferred), V
                    # scaling, and PE identity-accumulation — pipelined so PE
                    # aggregation of half 0 overlaps DVE scoring of half 1.
                    # The previous qblock's epilogue is emitted between the
                    # halves so its PSUM reads never stall this one's DVE.
                    E = smallp.tile([P, M, H], f16, tag="E")
                    psA = psump.tile([P, C], f32, tag="agg")
                    psZ = psump.tile([P, H], f32, tag="aggz")
                    emit_score(qb, 0, kvh2[0], A, E)
                    emit_score(qb, 1, kvh2[1], A, E)
                    emit_value(qb, 0, kvh2[0], E, psA, psZ)
                    if pend:
                        emit_epilogue(*pend.pop())
                    emit_value(qb, 1, kvh2[1], E, psA, psZ)
                    pend.append((qb, psA, psZ))
                emit_epilogue(*pend.pop())

    nc.compile()
    return nc


def _wrap_idx(lst):
    """int16 list -> [128, len/16] wrapped (pos i -> [i%16, i//16]) and
    replicated across the 8 groups of 16 partitions."""
    n = lst.shape[0]
    w = np.empty((P, n // 16), np.int16)
    blk = lst.reshape(n // 16, 16).T  # [16, n/16]
    for g in range(8):
        w[g * 16:(g + 1) * 16, :] = blk
    return w


def make_core_inputs(pairwise_g, coset_functions, nbhd_idx,
                     W_q, b_q, W_k, W_l, v, W_in, b_in, W_out, b_out):
    pairwise_g = np.asarray(pairwise_g)
    coset_functions = np.asarray(coset_functions)
    nbhd_idx = np.asarray(nbhd_idx).astype(np.int64)
    W_q = np.asarray(W_q, np.float32)
    b_q = np.asarray(b_q, np.float32)
    W_k = np.asarray(W_k, np.float32)
    W_l = np.asarray(W_l, np.float32)
    v = np.asarray(v, np.float32)
    W_in = np.asarray(W_in, np.float32)
    b_in = np.asarray(b_in, np.float32)
    W_out = np.asarray(W_out, np.float32)
    b_out = np.asarray(b_out, np.float32)

    wq16 = np.ascontiguousarray(
        W_q[CHMAP].T.reshape(2, P, C).astype(np.float16))
    wkv16 = np.ascontiguousarray(
        np.concatenate([W_k[CHMAP].T, W_in[CHMAP].T], axis=1)
        .reshape(2, P, CH2).astype(np.float16))
    wo16 = np.ascontiguousarray(
        W_out.T[CHMAP].reshape(2, P, C).astype(np.float16))
    wl_full = np.zeros((C, H * POS), np.float32)
    for h in range(H):
        for p_ in range(POS):
            wl_full[h * D:(h + 1) * D, p_ * H + h] = W_l[h * D:(h + 1) * D, p_]
    wl16 = np.ascontiguousarray(
        wl_full[CHMAP].reshape(2, P, H * POS).astype(np.float16))
    bqv32 = np.ascontiguousarray(
        (b_q + v.reshape(C))[CHMAP].reshape(2, P, 1).astype(np.float32))
    bqmat = np.ascontiguousarray(
        np.broadcast_to(b_q[CHMAP], (P, C)).astype(np.float32))
    bomat = np.ascontiguousarray(
        np.broadcast_to(W_out @ b_in + b_out, (P, C)).astype(np.float32))
    ident16 = np.eye(P, dtype=np.float16)

    in_maps = []
    for core in range(NCORES):
        b = core // 4
        qs = (core % 4) * NQ
        # roll keys so this core's queries are columns 0..NQ-1 of cosT
        # (the compiled program slices cosT[:, :, 0:NQ] for Q projections);
        # gather idx values are rotated to match the rolled KV table.
        cosT = np.ascontiguousarray(
            np.roll(coset_functions[b], -qs, axis=0)
            .T.reshape(2, P, N).astype(np.float16))
        idx = nbhd_idx[b, qs:qs + NQ]  # [NQ, M]
        idx_rot = (idx - qs) % N

        idxw = np.empty((P, QB, M * P // 16), np.int16)
        for qb in range(QB):
            blk = idx_rot[qb * P:(qb + 1) * P]  # [P(n), M]
            lst = blk.T.reshape(M * P)  # m-major: pos i = m*128 + n
            idxw[:, qb, :] = _wrap_idx(lst.astype(np.int16))

        # host-gathered pairwise_g neighbor rows, h-expanded, f16
        pg_rows = pairwise_g[b, qs + np.arange(NQ)[:, None], idx]  # [NQ,M,POS]
        pgx = np.broadcast_to(pg_rows[:, :, :, None].astype(np.float16),
                              (NQ, M, POS, H))
        pgx = np.ascontiguousarray(pgx).reshape(QB, P, M * POS * H)

        in_maps.append({
            "cosT": cosT,
            "wkv": wkv16, "wq": wq16, "wl": wl16, "wo": wo16,
            "bqv": bqv32, "bqmat": bqmat, "bomat": bomat,
            "ident": ident16, "idxw": idxw, "pgx": pgx,
        })
    return in_maps


def assemble_output(results):
    out = np.empty((B, N, C), np.float32)
    for core in range(NCORES):
        b = core // 4
        qs = (core % 4) * NQ
        o = results[core]["out"]  # [QB, P, C]
        out[b, qs:qs + NQ] = o.reshape(NQ, C)
    return out


def kernel(pairwise_g, coset_functions, mask, nbhd_idx,
           W_q, b_q, W_k, b_k, W_l, b_l, u, v,
           W_in, b_in, W_out, b_out, **_unused):
    from concourse.bass_utils import run_bass_kernel_spmd

    if "nc" not in _compiled:
        _compiled["nc"] = build_bass()
    nc = _compiled["nc"]

    in_maps = make_core_inputs(pairwise_g, coset_functions, nbhd_idx,
                               W_q, b_q, W_k, W_l, v, W_in, b_in,
                               W_out, b_out)
    res = run_bass_kernel_spmd(nc, in_maps, core_ids=list(range(NCORES)))
    return assemble_output(res.results)
